# revision 1
# baseline (speedup 1.0000x reference)
"""Trainium2 Bass kernel for nn_CausalSelfAttention (B=2, T=2048, C=1024, 16 heads).

Sharding: 8 cores = 2 batches x 4 head-groups (4 heads each). Each core:
  - computes Q^T/K^T ([d,t] layout) and V ([t,d]) for its heads from x^T
    (host pre-transposes x and pre-packs the weight slices in SBUF layout),
  - runs causal flash attention: S^T ([k,q]) via PE, exp on ScalarE with the
    1/sqrt(d) scale fused, then P@V with P^T as the *stationary* operand so
    the PV matmul runs at full PE rate ([q,d] output, 128-partition out),
    softmax denominators from a ones-column in V,
  - normalizes O in [q,d] layout (per-partition scalars), transposes O back
    to [d,q] via the DMA XBAR (off the PE critical path),
  - projects through its W_out row-slice producing a partial [T, C] output.
Out-projection blocks are interleaved into the attention loop as PE fillers;
a single set of tile pools is used throughout (no mid-kernel pool barriers).
Host sums the 4 tensor-parallel partials per batch (the "all-reduce") and
adds b_out.

Matmul inputs are bf16 (fp32 accumulation in PSUM).
"""
import sys

if '/opt/trn_rl_repo' not in sys.path:
    sys.path.insert(0, '/opt/trn_rl_repo')

import numpy as np
import ml_dtypes

B, T, C = 2, 2048, 1024
N_HEAD = 16
D = 64
P = 128
N_CORES = 8
GROUPS = N_CORES // B            # 4 tensor-parallel groups per batch
HPC = N_HEAD // GROUPS           # 4 heads per core
DH = HPC * D                     # 256 head dims per core
KO = C // P                      # 8 contraction subtiles for projections
NQB = T // 512                   # 4 q blocks of 512
SCALE = 1.0 / np.sqrt(D)
N_WARM = 12                       # PE warm-up matmuls while first loads land

_CACHE = {}


def _build():
    import concourse.mybir as mybir
    import concourse.tile as tile
    from concourse import bacc

    f32 = mybir.dt.float32
    bf16 = mybir.dt.bfloat16
    f16 = mybir.dt.float16

    nc = bacc.Bacc("TRN2", target_bir_lowering=False, debug=False,
                   num_devices=N_CORES)

    xt_d = nc.dram_tensor("xt", [C, T], bf16, kind="ExternalInput")
    wq_d = nc.dram_tensor("wq", [P, KO, DH], bf16, kind="ExternalInput")
    wk_d = nc.dram_tensor("wk", [P, KO, DH], bf16, kind="ExternalInput")
    wv_d = nc.dram_tensor("wv", [P, KO, DH], bf16, kind="ExternalInput")
    wo_d = nc.dram_tensor("wo", [P, 2, C], bf16, kind="ExternalInput")
    bq_d = nc.dram_tensor("bq", [P, 2], f32, kind="ExternalInput")
    bk_d = nc.dram_tensor("bk", [P, 2], f32, kind="ExternalInput")
    bv_d = nc.dram_tensor("bv", [1, DH], f32, kind="ExternalInput")
    tri_d = nc.dram_tensor("tri", [P, P], bf16, kind="ExternalInput")
    out_d = nc.dram_tensor("out", [T, C], f16, kind="ExternalOutput")

    EXP = mybir.ActivationFunctionType.Exp
    pls = [slice(0, D), slice(D, 2 * D)]

    with tile.TileContext(nc) as tc:
        with (
            tc.tile_pool(name="pp", bufs=1) as pp,
            tc.tile_pool(name="wB", bufs=32) as wB,
            tc.tile_pool(name="wS", bufs=4) as wS,
            tc.tile_pool(name="wO", bufs=4) as wO,
            tc.tile_pool(name="psS", bufs=2, space="PSUM") as psS,
            tc.tile_pool(name="psU", bufs=4, space="PSUM") as psU,
        ):
            xts = [pp.tile([P, KO, 512], bf16, tag=f"xt{q}", name=f"xt{q}")
                   for q in range(4)]
            wqs = pp.tile([P, KO, DH], bf16, tag="wqs")
            wks = pp.tile([P, KO, DH], bf16, tag="wks")
            wvs = pp.tile([P, KO, DH], bf16, tag="wvs")
            wos = pp.tile([P, 2, C], bf16, tag="wos")
            qts = [[pp.tile([P, 512], bf16, tag=f"qt{s}_{q}", name=f"qt{s}_{q}")
                    for q in range(4)] for s in range(2)]
            kts = [[pp.tile([P, 512], bf16, tag=f"kt{s}_{q}", name=f"kt{s}_{q}")
                    for q in range(4)] for s in range(2)]
            vos = [pp.tile([P, 4, HPC, D + 1], bf16, tag=f"vo{q}",
                           name=f"vo{q}") for q in range(4)]
            ots = [[pp.tile([P, 512], bf16, tag=f"ot{j}_{hs}",
                            name=f"ot{j}_{hs}") for hs in range(2)]
                   for j in range(NQB)]
            bqs = pp.tile([P, 2], f32, tag="bqs")
            bks = pp.tile([P, 2], f32, tag="bks")
            bvrow = pp.tile([1, DH], f32, tag="bvrow")
            bvb = pp.tile([P, DH], f32, tag="bvb")
            trib = pp.tile([P, P], bf16, tag="trib")
            warm = pp.tile([P, 512], bf16, tag="warm")

            # PE warm-up: junk matmuls on a zeroed tile keep the PE p-state
            # ramp going while the first input DMAs land.
            nc.vector.memset(warm[:], 0.0)
            for w in range(N_WARM):
                pw = psU.tile([P, 512], f32, tag="ps1", name=f"pw{w}")
                nc.tensor.matmul(pw[:], warm[:, 0:P], warm[:],
                                 start=True, stop=True)

            # ---- loads (SP + ACT HWDGE queues in parallel at startup) ----
            xt_r = xt_d.rearrange("(ko p) t -> p ko t", p=P)
            nc.sync.dma_start(wqs[:, :, 0:P], wq_d[:, :, 0:P])
            nc.scalar.dma_start(xts[0][:, :, 0:256], xt_r[:, :, 0:256])
            nc.sync.dma_start(bqs[:], bq_d[:])
            nc.sync.dma_start(wqs[:, :, P:2 * P], wq_d[:, :, P:2 * P])
            nc.scalar.dma_start(xts[0][:, :, 256:512], xt_r[:, :, 256:512])
            nc.sync.dma_start(bks[:], bk_d[:])
            nc.sync.dma_start(trib[:], tri_d[:])
            nc.sync.dma_start(bvrow[:], bv_d[:])
            nc.sync.dma_start(wks[:], wk_d[:])
            nc.sync.dma_start(xts[1][:], xt_r[:, :, 512:1024])
            nc.sync.dma_start(wvs[:], wv_d[:])
            nc.sync.dma_start(xts[2][:], xt_r[:, :, 1024:1536])
            nc.sync.dma_start(xts[3][:], xt_r[:, :, 1536:2048])
            nc.sync.dma_start(wos[:], wo_d[:])

            nc.gpsimd.partition_broadcast(bvb[:, :], bvrow[0:1, :])
            for q in range(4):
                nc.vector.memset(vos[q][:, :, :, D:D + 1], 1.0)
            # trigger the exp ACT-table load early
            scr = pp.tile([1, 1], f32, tag="scr")
            nc.scalar.activation(scr[0:1, 0:1], trib[0:1, 0:1], EXP)

            # ---- emit helpers (each *_items returns a list of thunks; one
            # thunk = one contiguous chunk of PE work) ----
            def qk_items(q, chunks):
                items = []
                for wsb, dsts, bias in ((wqs, qts, bqs), (wks, kts, bks)):
                    for s_ in range(2):
                        for (lo, hi) in chunks:
                            def it(wsb=wsb, dsts=dsts, bias=bias, s_=s_,
                                   lo=lo, hi=hi):
                                w_ = hi - lo
                                pq = psU.tile([P, 512], f32, tag="ps1",
                                              name=f"pq{q}_{s_}_{lo}")
                                for ko in range(KO):
                                    nc.tensor.matmul(
                                        pq[:, 0:w_],
                                        wsb[:, ko, s_ * P:(s_ + 1) * P],
                                        xts[q][:, ko, lo:hi],
                                        start=(ko == 0), stop=(ko == KO - 1))
                                nc.vector.tensor_scalar_add(
                                    dsts[s_][q][:, lo:hi], pq[:, 0:w_],
                                    bias[:, s_:s_ + 1])
                            items.append(it)
                return items

            def v_items(q):
                items = []
                for it_ in range(4):
                    def it(it_=it_):
                        pv = psU.tile([P, 512], f32, tag="ps1",
                                      name=f"pv{q}_{it_}")
                        for ko in range(KO):
                            nc.tensor.matmul(
                                pv[:, 0:DH],
                                xts[q][:, ko, it_ * P:(it_ + 1) * P],
                                wvs[:, ko, :],
                                start=(ko == 0), stop=(ko == KO - 1))
                        nc.vector.tensor_tensor(
                            vos[q][:, it_, :, 0:D],
                            pv[:, 0:DH].rearrange("p (h d) -> p h d", h=HPC),
                            bvb.rearrange("p (h d) -> p h d", h=HPC),
                            mybir.AluOpType.add)
                    items.append(it)
                return items

            def emit_s_tile(q, hs, i, pts):
                off = max(0, P * i - 512 * q)
                sp = psS.tile([P, 2, 512], f32, tag="sp",
                              name=f"sp{q}_{hs}_{i}")
                pt = wB.tile([P, 2, 512], bf16, tag="pt",
                             name=f"pt{q}_{hs}_{i}")
                for u in range(2):
                    nc.tensor.matmul(
                        sp[:, u, off:512],
                        kts[hs][i // 4][pls[u],
                                        (i % 4) * P:(i % 4 + 1) * P],
                        qts[hs][q][pls[u], off:512],
                        start=True, stop=True)
                nc.scalar.activation(pt[:, :, off:512],
                                     sp[:, :, off:512],
                                     EXP, scale=float(SCALE))
                if P * i >= 512 * q:  # diagonal triangle
                    for u in range(2):
                        nc.vector.tensor_mul(
                            pt[:, u, off:off + P],
                            pt[:, u, off:off + P], trib[:])
                pts.append(pt)

            def emit_pv(q, hs, pts, qcs):
                for qc in qcs:
                    qt = 4 * q + qc
                    # one PSUM bank per head (zero-region = 2KB: only one
                    # accumulation group may live in a bank at a time)
                    po = [psU.tile([P, 512], f32, tag="ps1",
                                   name=f"po{q}_{hs}_{qc}_{u}")
                          for u in range(2)]
                    for u in range(2):
                        for kt in range(qt + 1):
                            nc.tensor.matmul(
                                po[u][:, 0:D + 1],
                                pts[kt][:, u, qc * P:(qc + 1) * P],
                                vos[kt // 4][:, kt % 4, 2 * hs + u, :],
                                start=(kt == 0), stop=(kt == qt))
                    rd = wS.tile([P, 2], f32, tag="rd",
                                 name=f"rd{q}_{hs}_{qc}")
                    osb = wS.tile([P, 2, D], bf16, tag="osb",
                                  name=f"osb{q}_{hs}_{qc}")
                    for u in range(2):
                        nc.vector.reciprocal_approx_fast(
                            rd[:, u:u + 1], po[u][:, D:D + 1])
                        nc.vector.tensor_scalar_mul(
                            osb[:, u, :], po[u][:, 0:D], rd[:, u:u + 1])
                    nc.sync.dma_start(
                        ots[q][hs][:, qc * P:(qc + 1) * P], osb[:],
                        transpose=True)

            def c_items(jj, mos):
                # one [P, C] staging tile + ONE store per row-block, issued
                # through the software DGE on the idle GpSimd engine so the
                # stores never contend with loads/transposes on HWDGE
                items = []
                obs = {}
                for mo in mos:
                    for n in range(2):
                        def it(mo=mo, n=n):
                            m = 4 * jj + mo
                            pc = psU.tile([P, 512], f32, tag="ps1",
                                          name=f"pc{jj}_{mo}_{n}")
                            for s in range(2):
                                nc.tensor.matmul(
                                    pc[:],
                                    ots[jj][s][:, mo * P:(mo + 1) * P],
                                    wos[:, s, n * 512:(n + 1) * 512],
                                    start=(s == 0), stop=(s == 1))
                            if n == 0:
                                obs[mo] = wO.tile([P, C], f16, tag="ob",
                                                  name=f"ob{jj}_{mo}")
                            ob = obs[mo]
                            nc.vector.tensor_copy(
                                ob[:, n * 512:(n + 1) * 512], pc[:])
                            if n == 1:
                                nc.gpsimd.dma_start(
                                    out_d[m * P:(m + 1) * P, :], ob[:])
                        items.append(it)
                return items

            # ---- main pipeline ----
            # Per iteration q: spread the S i-tiles (whose exps are the ACT
            # bottleneck) among filler PE work (prev block's out-projection,
            # part of the next quarter's projections); the rest of the A
            # segment sits between PV(q,0) and PV(q,1) so the second head
            # pair's exps can drain before PV(q,1) consumes them.
            def spread(tiles, fillers):
                # emit S tiles with fillers distributed evenly between them
                nS, nF = len(tiles), len(fillers)
                fi = 0
                for k, t in enumerate(tiles, 1):
                    t()
                    while fi < (k * nF) // nS:
                        fillers[fi]()
                        fi += 1

            for it in qk_items(0, [(0, 256), (256, 512)]) + v_items(0):
                it()
            carry = []
            for q in range(4):
                fillA = (qk_items(q + 1, [(0, 512)]) + v_items(q + 1)
                         if q < 3 else [])
                if q == 2:
                    # V(3) moves to iteration 3's filler pool where the ACT
                    # exp backlog is binding; exp(2,1) has enough drain time
                    # without a mid segment here.
                    carry = fillA[4:]
                    fillA = fillA[0:4] + []
                fillC = c_items(q - 1, [0, 1, 2, 3]) if q > 0 else []
                if q == 3:
                    fillC = fillC + carry
                pts = [[], []]
                pv0 = [lambda qc=qc: emit_pv(q, 0, pts[0], [qc])
                       for qc in range(4)]
                s0 = [lambda i=i: emit_s_tile(q, 0, i, pts[0])
                      for i in range(4 * (q + 1))]
                s1 = [lambda i=i: emit_s_tile(q, 1, i, pts[1])
                      for i in range(4 * (q + 1))]
                # S tiles spread with C(q-1) + part of A(q+1) as fillers;
                # then PV(q,0), the rest of the A segment (so exp(q,1)
                # drains), then PV(q,1)
                spread(s0 + s1, fillC + fillA[0:4])
                for it in pv0:
                    it()
                for it in fillA[4:]:
                    it()
                if q < 3:
                    emit_pv(q, 1, pts[1], [0, 1, 2, 3])
                else:
                    # tail: stagger the final out-projection one q-tile
                    # behind PV so the transpose chain latency is hidden
                    cpend = []
                    for qc in range(4):
                        emit_pv(3, 1, pts[1], [qc])
                        if qc >= 1:
                            cpend.extend(c_items(3, [qc - 1]))
                        if len(cpend) >= 2:
                            cpend.pop(0)()
                            cpend.pop(0)()
                    for it in cpend + c_items(3, [3]):
                        it()

    nc.compile()
    return nc


def _get_nc():
    if "nc" not in _CACHE:
        _CACHE["nc"] = _build()
    return _CACHE["nc"]


def _get_runner():
    """Build the jitted SPMD executor once (mirrors bass2jax.run_bass_via_pjrt
    but caches the jitted function so repeat calls skip retrace/recompile)."""
    if "runner" in _CACHE:
        return _CACHE["runner"]
    import jax
    import numpy as _np
    from jax.sharding import Mesh, PartitionSpec
    from jax.experimental.shard_map import shard_map
    import concourse.mybir as mybir
    from concourse import bass2jax

    nc = _get_nc()
    bass2jax.install_neuronx_cc_hook()

    partition_name = (nc.partition_id_tensor.name
                      if nc.partition_id_tensor else None)
    in_names, out_names, out_avals, zero_shapes = [], [], [], []
    for alloc in nc.m.functions[0].allocations:
        if not isinstance(alloc, mybir.MemoryLocationSet):
            continue
        name = alloc.memorylocations[0].name
        if alloc.kind == "ExternalInput":
            if name != partition_name:
                in_names.append(name)
        elif alloc.kind == "ExternalOutput":
            out_avals.append(jax.core.ShapedArray(
                tuple(alloc.tensor_shape), mybir.dt.np(alloc.dtype)))
            out_names.append(name)
            zero_shapes.append((tuple(alloc.tensor_shape),
                                mybir.dt.np(alloc.dtype)))
    n_params = len(in_names)
    n_outs = len(out_names)
    all_names = in_names + out_names
    if partition_name is not None:
        all_names = all_names + [partition_name]

    def _body(*args):
        operands = list(args)
        if partition_name is not None:
            operands.append(bass2jax.partition_id_tensor())
        outs = bass2jax._bass_exec_p.bind(
            *operands,
            out_avals=tuple(out_avals),
            in_names=tuple(all_names),
            out_names=tuple(out_names),
            lowering_input_output_aliases=(),
            sim_require_finite=True,
            sim_require_nnan=True,
            nc=nc,
        )
        return tuple(outs)

    devices = jax.devices()[:N_CORES]
    mesh = Mesh(_np.asarray(devices), ("core",))
    donate = tuple(range(n_params, n_params + n_outs))
    sharded = jax.jit(
        shard_map(_body, mesh=mesh,
                  in_specs=(PartitionSpec("core"),) * (n_params + n_outs),
                  out_specs=(PartitionSpec("core"),) * n_outs,
                  check_rep=False),
        donate_argnums=donate, keep_unused=True)

    def run(in_maps):
        concat_in = [
            _np.concatenate([_np.asarray(m[name]) for m in in_maps], axis=0)
            for name in in_names]
        concat_zeros = [
            _np.zeros((N_CORES * sh[0], *sh[1:]), dtype)
            for sh, dtype in zero_shapes]
        out_arrs = sharded(*concat_in, *concat_zeros)
        return [
            {name: _np.asarray(out_arrs[i]).reshape(
                N_CORES, *zero_shapes[i][0])[c]
             for i, name in enumerate(out_names)}
            for c in range(N_CORES)]

    _CACHE["runner"] = run
    return run


def kernel(x, mask, W_qkv, b_qkv, W_out, b_out):
    bf = ml_dtypes.bfloat16
    x = np.asarray(x, dtype=np.float32)
    W_qkv = np.asarray(W_qkv, dtype=np.float32)
    b_qkv = np.asarray(b_qkv, dtype=np.float32)
    W_out = np.asarray(W_out, dtype=np.float32)
    b_out = np.asarray(b_out, dtype=np.float32)
    # mask is the causal tril mask (per problem spec); causality is
    # implemented structurally on-device.

    run = _get_runner()

    def pack_w(wslice):
        # [C, DH] -> [P, KO, DH] with C = ko*P + p
        return np.ascontiguousarray(
            wslice.reshape(KO, P, DH).transpose(1, 0, 2)).astype(bf)

    def pack_b(bslice):
        # [DH] -> [P, 2] with idx = s*P + p
        return np.ascontiguousarray(
            bslice.reshape(2, P).T).astype(np.float32)

    tri = np.triu(np.ones((P, P), dtype=np.float32)).astype(bf)

    xts = [np.ascontiguousarray(x[b].T).astype(bf) for b in range(B)]
    in_maps = []
    for core in range(N_CORES):
        b, g = divmod(core, GROUPS)
        cs = slice(g * DH, (g + 1) * DH)
        in_maps.append({
            "xt": xts[b],
            "wq": pack_w(W_qkv[:, cs]),
            "wk": pack_w(W_qkv[:, C:][:, cs]),
            "wv": pack_w(W_qkv[:, 2 * C:][:, cs]),
            "wo": np.ascontiguousarray(
                W_out[cs, :].reshape(2, P, C).transpose(1, 0, 2)).astype(bf),
            "bq": pack_b(b_qkv[cs]),
            "bk": pack_b(b_qkv[C:][cs]),
            "bv": np.ascontiguousarray(
                b_qkv[2 * C:][cs][None, :]).astype(np.float32),
            "tri": tri,
        })

    results = run(in_maps)

    out = np.zeros((B, T, C), dtype=np.float32)
    for core in range(N_CORES):
        b = core // GROUPS
        out[b] += results[core]["out"].astype(np.float32)
    out += b_out[None, None, :]
    return out



# revision 17
# speedup vs baseline: 1.0397x; 1.0397x over previous
"""Trainium2 Bass kernel for nn_CausalSelfAttention (B=2, T=2048, C=1024, 16 heads).

Sharding: 8 cores = 2 batches x 4 head-groups (4 heads each).

v3 design (cost-model driven):
  - QKV projections run as fp8(e4m3) DoubleRow matmuls with hi/lo error
    compensation: 32*x@W = xh@fp8(32W) + fp8(32(x-xh))@fp8(W)
    + xh@fp8(32W - fp8(32W)), all three terms accumulated in one PSUM
    group. DR charges 0.5 cycles/output-col and packs 2 contraction
    chunks per instruction, so this costs 0.75x of bf16 at ~0.3% error
    (plain fp8 would be 0.25x cost but ~5% error -- softmax averaging
    shrinks signal and noise equally, so that error hits the output
    full-strength and blows the 2e-2 gate).
  - S = Q^T K, P (exp output), P@V, and the out-projection stay bf16.
  - exp runs on ACT (~73us busy); PE (~90us busy) is the bottleneck, the
    emission weaves S tiles and filler work by a cost ledger.
  - diagonal causal masking: post-exp multiply by triu-ones on the
    (otherwise idle) gpsimd/Pool engine.
  - O is normalized in [q,d] layout then transposed via PE (identity
    matmul) + DVE copy -- no DMA XBAR on the critical path.
  - out-projection in bf16, partial [T,C] per core; host sums the 4
    tensor-parallel partials per batch and adds b_out.
"""
import sys

if '/opt/trn_rl_repo' not in sys.path:
    sys.path.insert(0, '/opt/trn_rl_repo')

import numpy as np
import ml_dtypes

B, T, C = 2, 2048, 1024
N_HEAD = 16
D = 64
P = 128
N_CORES = 8
GROUPS = N_CORES // B            # 4 tensor-parallel groups per batch
HPC = N_HEAD // GROUPS           # 4 heads per core
DH = HPC * D                     # 256 head dims per core
KO = C // P                      # 8 contraction subtiles for projections
NQB = T // 512                   # 4 q blocks of 512
WSCALE = 32.0                    # fp8 range scaling of W_qkv
SCALE = 1.0 / (np.sqrt(D) * WSCALE * WSCALE)   # exp scale

_CACHE = {}

# weave pacing costs (ns)
PE_CYC = 0.4167


def _build():
    import concourse.mybir as mybir
    import concourse.tile as tile
    from concourse import bacc

    f32 = mybir.dt.float32
    bf16 = mybir.dt.bfloat16
    f16 = mybir.dt.float16
    fp8 = mybir.dt.float8e4
    DR = mybir.MatmulPerfMode.DoubleRow
    EXP = mybir.ActivationFunctionType.Exp
    MUL = mybir.AluOpType.mult
    ADD = mybir.AluOpType.add

    nc = bacc.Bacc("TRN2", target_bir_lowering=False, debug=False,
                   num_devices=N_CORES)

    xh_d = nc.dram_tensor("xh", [C, T], fp8, kind="ExternalInput")
    xl_d = nc.dram_tensor("xl", [C, T], fp8, kind="ExternalInput")
    w_ds = {}
    for nm in ("wqh", "wq1", "wql", "wkh", "wk1", "wkl",
               "wvh", "wv1", "wvl"):
        w_ds[nm] = nc.dram_tensor(nm, [P, KO, DH], fp8, kind="ExternalInput")
    wo_d = nc.dram_tensor("wo", [P, 2, C], bf16, kind="ExternalInput")
    bq_d = nc.dram_tensor("bq", [P, 2], f32, kind="ExternalInput")
    bk_d = nc.dram_tensor("bk", [P, 2], f32, kind="ExternalInput")
    bv_d = nc.dram_tensor("bv", [1, DH], f32, kind="ExternalInput")
    tri_d = nc.dram_tensor("tri", [P, 2, P], bf16, kind="ExternalInput")
    id_d = nc.dram_tensor("ident", [P, P], bf16, kind="ExternalInput")
    out_d = nc.dram_tensor("out", [T, C], f16, kind="ExternalOutput")

    pls = [slice(0, D), slice(D, 2 * D)]

    with tile.TileContext(nc) as tc:
        with (
            tc.tile_pool(name="pp", bufs=1) as pp,
            tc.tile_pool(name="wS", bufs=4) as wS,
            tc.tile_pool(name="wO", bufs=4) as wO,
            tc.tile_pool(name="psS", bufs=2, space="PSUM") as psS,
            tc.tile_pool(name="psU", bufs=4, space="PSUM") as psU,
        ):
            xhs = [pp.tile([P, KO, 512], fp8, tag=f"xh{q}", name=f"xh{q}")
                   for q in range(4)]
            xls = [pp.tile([P, KO, 512], fp8, tag=f"xl{q}", name=f"xl{q}")
                   for q in range(4)]
            wt = {nm: pp.tile([P, KO, DH], fp8, tag=nm, name=nm)
                  for nm in ("wqh", "wq1", "wql", "wkh", "wk1", "wkl",
                             "wvh", "wv1", "wvl")}
            wos = pp.tile([P, 2, C], bf16, tag="wos")
            qts = [[pp.tile([P, 512], bf16, tag=f"qt{s}_{q}", name=f"qt{s}_{q}")
                    for q in range(4)] for s in range(2)]
            kts = [[pp.tile([P, 512], bf16, tag=f"kt{s}_{q}", name=f"kt{s}_{q}")
                    for q in range(4)] for s in range(2)]
            # vo: [k-part, kt-slot, head(2hs+u), d + ones-col]
            vo = pp.tile([P, 16, HPC, D + 1], bf16, tag="vo")
            # pt: per hs P^T tiles [k-part, kt-slot, u, q-col of current block]
            pts = [pp.tile([P, 16, 2, 512], bf16, tag=f"pt{hs}", name=f"pt{hs}")
                   for hs in range(2)]
            ots = [[pp.tile([P, 512], bf16, tag=f"ot{j}_{hs}",
                            name=f"ot{j}_{hs}") for hs in range(2)]
                   for j in range(NQB)]
            bqs = pp.tile([P, 2], f32, tag="bqs")
            bks = pp.tile([P, 2], f32, tag="bks")
            bvrow = pp.tile([1, DH], f32, tag="bvrow")
            bvb = pp.tile([P, DH], f32, tag="bvb")
            trib = pp.tile([P, 2, P], bf16, tag="trib")
            ident = pp.tile([P, P], bf16, tag="ident")
            scr = pp.tile([1, 1], f32, tag="scr")

            # trigger the exp ACT-table load immediately (scratch memset)
            nc.vector.memset(scr[:], 0.0)
            nc.scalar.activation(scr[0:1, 0:1], scr[0:1, 0:1], EXP)
            # ones-column (=WSCALE) of V for softmax denominators
            nc.vector.memset(vo[:, :, :, D:D + 1], WSCALE)

            # ---- loads (sync=SP queue, scalar=ACT queue) ----
            xh_r = xh_d.rearrange("(ko p) t -> p ko t", p=P)
            xl_r = xl_d.rearrange("(ko p) t -> p ko t", p=P)
            for nm in ("wqh", "wq1", "wql", "wkh", "wk1", "wkl"):
                nc.sync.dma_start(wt[nm][:], w_ds[nm][:])
            nc.scalar.dma_start(xhs[0][:], xh_r[:, :, 0:512])
            nc.scalar.dma_start(xls[0][:], xl_r[:, :, 0:512])
            nc.sync.dma_start(bqs[:], bq_d[:])
            nc.sync.dma_start(bks[:], bk_d[:])
            nc.sync.dma_start(trib[:], tri_d[:])
            nc.sync.dma_start(ident[:], id_d[:])
            nc.sync.dma_start(bvrow[:], bv_d[:])
            nc.scalar.dma_start(xhs[1][:], xh_r[:, :, 512:1024])
            nc.scalar.dma_start(xls[1][:], xl_r[:, :, 512:1024])
            for nm in ("wvh", "wv1", "wvl"):
                nc.sync.dma_start(wt[nm][:], w_ds[nm][:])
            nc.scalar.dma_start(xhs[2][:], xh_r[:, :, 1024:1536])
            nc.scalar.dma_start(xls[2][:], xl_r[:, :, 1024:1536])
            nc.scalar.dma_start(xhs[3][:], xh_r[:, :, 1536:2048])
            nc.scalar.dma_start(xls[3][:], xl_r[:, :, 1536:2048])
            nc.sync.dma_start(wos[:], wo_d[:])

            nc.gpsimd.partition_broadcast(bvb[:, :], bvrow[0:1, :])

            # ---------- item constructors (thunk, pe_ns, act_ns) ----------
            # hi/lo fp8 compensation: 32xW = xh@Wh + xl@W1 + xh@Wl, all
            # accumulated in one PSUM group (12 DR steps).
            def qk8_item(q, wpfx, dsts, bias, s_):
                terms = [(wt[wpfx + "h"], xhs), (wt[wpfx + "1"], xls),
                         (wt[wpfx + "l"], xhs)]

                def it():
                    pq = psU.tile([P, 512], f32, tag="ps1",
                                  name=f"pq{q}_{wpfx}_{s_}")
                    for ti, (w8, xsrc) in enumerate(terms):
                        for t in range(KO // 2):
                            nc.tensor.matmul(
                                pq[:],
                                w8[:, 2 * t:2 * t + 2, s_ * P:(s_ + 1) * P],
                                xsrc[q][:, 2 * t:2 * t + 2, :],
                                start=(ti == 0 and t == 0),
                                stop=(ti == 2 and t == KO // 2 - 1),
                                perf_mode=DR)
                    nc.vector.tensor_scalar_add(
                        dsts[s_][q][:], pq[:], bias[:, s_:s_ + 1])
                return (it, 1290, 0)

            def qk8_items(q):
                # order: Q s0, K s0, Q s1, K s1 (heads-split 0 first so the
                # next phase's S(q,0) stream unblocks earliest)
                return [qk8_item(q, "wq", qts, bqs, 0),
                        qk8_item(q, "wk", kts, bks, 0),
                        qk8_item(q, "wq", qts, bqs, 1),
                        qk8_item(q, "wk", kts, bks, 1)]

            def v8_item(q, it_):
                terms = [(wt["wvh"], xhs), (wt["wv1"], xls),
                         (wt["wvl"], xhs)]

                def it():
                    pv = psU.tile([P, DH], f32, tag="ps1",
                                  name=f"pv{q}_{it_}")
                    for ti, (w8, xsrc) in enumerate(terms):
                        for t in range(KO // 2):
                            nc.tensor.matmul(
                                pv[:],
                                xsrc[q][:, 2 * t:2 * t + 2,
                                        it_ * P:(it_ + 1) * P],
                                w8[:, 2 * t:2 * t + 2, :],
                                start=(ti == 0 and t == 0),
                                stop=(ti == 2 and t == KO // 2 - 1),
                                perf_mode=DR)
                    nc.vector.tensor_tensor(
                        vo[:, 4 * q + it_, :, 0:D],
                        pv[:].rearrange("p (h d) -> p h d", h=HPC),
                        bvb.rearrange("p (h d) -> p h d", h=HPC),
                        ADD)
                return (it, 645, 0)

            def s_item(q, hs, i):
                off = max(0, P * i - 512 * q)
                diag = P * i >= 512 * q

                def it():
                    sp = psS.tile([P, 2, 512], f32, tag="sp",
                                  name=f"sp{q}_{hs}_{i}")
                    for u in range(2):
                        nc.tensor.matmul(
                            sp[:, u, off:512],
                            kts[hs][i // 4][pls[u],
                                            (i % 4) * P:(i % 4 + 1) * P],
                            qts[hs][q][pls[u], off:512],
                            start=True, stop=True)
                    nc.scalar.activation(pts[hs][:, i, :, off:512],
                                         sp[:, :, off:512],
                                         EXP, scale=float(SCALE))
                    if diag:
                        nc.gpsimd.tensor_tensor(
                            pts[hs][:, i, :, off:off + P],
                            pts[hs][:, i, :, off:off + P],
                            trib[:], MUL)
                w = 512 - off
                return (it, 2 * w * PE_CYC + 20, 2 * w * 0.833 + 190)

            def pv_item(q, hs, qc, u):
                qt = 4 * q + qc

                def it():
                    po = psU.tile([P, 512], f32, tag="ps1",
                                  name=f"po{q}_{hs}_{qc}_{u}")
                    for kt in range(qt + 1):
                        nc.tensor.matmul(
                            po[:, 0:D + 1],
                            pts[hs][:, kt, u, qc * P:(qc + 1) * P],
                            vo[:, kt, 2 * hs + u, :],
                            start=(kt == 0), stop=(kt == qt))
                    rd = wS.tile([P, 1], f32, tag="rd",
                                 name=f"rd{q}_{hs}_{qc}_{u}")
                    if u == 0:
                        osbs[(q, hs, qc)] = wS.tile(
                            [P, 2, D], bf16, tag="osb",
                            name=f"osb{q}_{hs}_{qc}")
                    osb = osbs[(q, hs, qc)]
                    nc.vector.reciprocal_approx_fast(rd[:], po[:, D:D + 1])
                    nc.vector.tensor_scalar_mul(
                        osb[:, u, :], po[:, 0:D], rd[:])
                return (it, (qt + 1) * 27 + 30, 0)

            osbs = {}

            def tr_item(q, hs, qc):
                def it():
                    ptr = psU.tile([P, P], bf16, tag="ps1",
                                   name=f"ptr{q}_{hs}_{qc}")
                    nc.tensor.transpose(
                        ptr[:], osbs[(q, hs, qc)].rearrange("p u d -> p (u d)"),
                        ident[:])
                    nc.vector.tensor_copy(
                        ots[q][hs][:, qc * P:(qc + 1) * P], ptr[:])
                return (it, 75, 0)

            def c_item(jj, mo, n):
                def it():
                    pc = psU.tile([P, 512], f32, tag="ps1",
                                  name=f"pc{jj}_{mo}_{n}")
                    for s in range(2):
                        nc.tensor.matmul(
                            pc[:],
                            ots[jj][s][:, mo * P:(mo + 1) * P],
                            wos[:, s, n * 512:(n + 1) * 512],
                            start=(s == 0), stop=(s == 1))
                    if n == 0:
                        obs[(jj, mo)] = wO.tile([P, C], f16, tag="ob",
                                                name=f"ob{jj}_{mo}")
                    ob = obs[(jj, mo)]
                    nc.vector.tensor_copy(
                        ob[:, n * 512:(n + 1) * 512], pc[:])
                    if n == 1:
                        m = 4 * jj + mo
                        nc.gpsimd.dma_start(
                            out_d[m * P:(m + 1) * P, :], ob[:])
                return (it, 430, 0)

            obs = {}

            # ---------- schedule ----------
            # master stream: all S tiles in (q, hs, i) order; ACT must never
            # starve. fillers paced by a PE-vs-ACT cost ledger.
            def weave(masters, fillers):
                act_t = 0.0
                pe_t = 0.0
                fi = 0
                for (it, pe, act) in masters:
                    it()
                    act_t += act
                    pe_t += pe
                    # keep PE fed while ACT is ahead; +1 filler lookahead
                    while fi < len(fillers) and pe_t + 400 < act_t:
                        f, fpe, _ = fillers[fi]
                        f()
                        pe_t += fpe
                        fi += 1
                # drain leftovers
                while fi < len(fillers):
                    f, fpe, _ = fillers[fi]
                    f()
                    fi += 1

            # prologue: projections for q=0 (heads-split 0 first so S can
            # start after 2 items)
            pro = qk8_items(0)
            pro[0][0]()  # Q s=0
            pro[1][0]()  # K s=0
            masters0 = [s_item(0, 0, i) for i in range(4)]
            fillers0 = [pro[2], pro[3],  # Q s1, K s1
                        v8_item(0, 0), v8_item(0, 1),
                        v8_item(0, 2), v8_item(0, 3)]
            weave(masters0, fillers0)

            for q in range(4):
                # segment A: S(q, 1) stream (for q=0 handled below with
                # fillers PV(q,0)); segments are:
                #   S(q, 0): fillers = PV(q-1,1,*), tr(q-1,1,*), A(q+1) pt1
                #   S(q, 1): fillers = PV(q,0,*), tr(q,0,*), c(q-1),
                #            A(q+1) pt2, v8(q+1)
                if q > 0:
                    mastersA = [s_item(q, 0, i) for i in range(4 * q + 4)]
                    fillersA = []
                    for qc in range(4):
                        fillersA.append(pv_item(q - 1, 1, qc, 0))
                        fillersA.append(pv_item(q - 1, 1, qc, 1))
                        if qc >= 1:
                            fillersA.append(tr_item(q - 1, 1, qc - 1))
                    fillersA.append(tr_item(q - 1, 1, 3))
                    if q < 3:
                        for it_ in qk8_items(q + 1)[0:2]:  # Q s0, K s0
                            fillersA.append(it_)
                    weave(mastersA, fillersA)

                mastersB = [s_item(q, 1, i) for i in range(4 * q + 4)]
                fillersB = []
                for qc in range(4):
                    fillersB.append(pv_item(q, 0, qc, 0))
                    fillersB.append(pv_item(q, 0, qc, 1))
                    if qc >= 1:
                        fillersB.append(tr_item(q, 0, qc - 1))
                fillersB.append(tr_item(q, 0, 3))
                if q > 0:
                    for mo in range(4):
                        fillersB.append(c_item(q - 1, mo, 0))
                        fillersB.append(c_item(q - 1, mo, 1))
                if q < 3:
                    nxt = qk8_items(q + 1)
                    # segment A of phase q (absent for q=0) carried s0 items
                    for it_ in (nxt if q == 0 else nxt[2:4]):
                        fillersB.append(it_)
                    for it_ in [v8_item(q + 1, k) for k in range(4)]:
                        fillersB.append(it_)
                weave(mastersB, fillersB)

            # tail: PV(3,1,*) + transposes + c(3,*) staggered
            for qc in range(4):
                pv_item(3, 1, qc, 0)[0]()
                pv_item(3, 1, qc, 1)[0]()
                if qc >= 1:
                    tr_item(3, 1, qc - 1)[0]()
                if qc >= 2:
                    c_item(3, qc - 2, 0)[0]()
                    c_item(3, qc - 2, 1)[0]()
            tr_item(3, 1, 3)[0]()
            for mo in (2, 3):
                c_item(3, mo, 0)[0]()
                c_item(3, mo, 1)[0]()

    nc.compile()
    return nc


def _get_nc():
    if "nc" not in _CACHE:
        _CACHE["nc"] = _build()
    return _CACHE["nc"]


def _get_runner():
    """Build the jitted SPMD executor once (mirrors bass2jax.run_bass_via_pjrt
    but caches the jitted function so repeat calls skip retrace/recompile)."""
    if "runner" in _CACHE:
        return _CACHE["runner"]
    import jax
    import numpy as _np
    from jax.sharding import Mesh, PartitionSpec
    from jax.experimental.shard_map import shard_map
    import concourse.mybir as mybir
    from concourse import bass2jax

    nc = _get_nc()
    bass2jax.install_neuronx_cc_hook()

    partition_name = (nc.partition_id_tensor.name
                      if nc.partition_id_tensor else None)
    in_names, out_names, out_avals, zero_shapes = [], [], [], []
    for alloc in nc.m.functions[0].allocations:
        if not isinstance(alloc, mybir.MemoryLocationSet):
            continue
        name = alloc.memorylocations[0].name
        if alloc.kind == "ExternalInput":
            if name != partition_name:
                in_names.append(name)
        elif alloc.kind == "ExternalOutput":
            out_avals.append(jax.core.ShapedArray(
                tuple(alloc.tensor_shape), mybir.dt.np(alloc.dtype)))
            out_names.append(name)
            zero_shapes.append((tuple(alloc.tensor_shape),
                                mybir.dt.np(alloc.dtype)))
    n_params = len(in_names)
    n_outs = len(out_names)
    all_names = in_names + out_names
    if partition_name is not None:
        all_names = all_names + [partition_name]

    def _body(*args):
        operands = list(args)
        if partition_name is not None:
            operands.append(bass2jax.partition_id_tensor())
        outs = bass2jax._bass_exec_p.bind(
            *operands,
            out_avals=tuple(out_avals),
            in_names=tuple(all_names),
            out_names=tuple(out_names),
            lowering_input_output_aliases=(),
            sim_require_finite=True,
            sim_require_nnan=True,
            nc=nc,
        )
        return tuple(outs)

    devices = jax.devices()[:N_CORES]
    mesh = Mesh(_np.asarray(devices), ("core",))
    donate = tuple(range(n_params, n_params + n_outs))
    sharded = jax.jit(
        shard_map(_body, mesh=mesh,
                  in_specs=(PartitionSpec("core"),) * (n_params + n_outs),
                  out_specs=(PartitionSpec("core"),) * n_outs,
                  check_rep=False),
        donate_argnums=donate, keep_unused=True)

    def run(in_maps):
        concat_in = [
            _np.concatenate([_np.asarray(m[name]) for m in in_maps], axis=0)
            for name in in_names]
        concat_zeros = [
            _np.zeros((N_CORES * sh[0], *sh[1:]), dtype)
            for sh, dtype in zero_shapes]
        out_arrs = sharded(*concat_in, *concat_zeros)
        return [
            {name: _np.asarray(out_arrs[i]).reshape(
                N_CORES, *zero_shapes[i][0])[c]
             for i, name in enumerate(out_names)}
            for c in range(N_CORES)]

    _CACHE["runner"] = run
    return run


def _fp8():
    return (ml_dtypes.float8_e4m3fn if hasattr(ml_dtypes, 'float8_e4m3fn')
            else ml_dtypes.float8_e4m3)


def kernel(x, mask, W_qkv, b_qkv, W_out, b_out):
    bf = ml_dtypes.bfloat16
    f8 = _fp8()
    x = np.asarray(x, dtype=np.float32)
    W_qkv = np.asarray(W_qkv, dtype=np.float32)
    b_qkv = np.asarray(b_qkv, dtype=np.float32)
    W_out = np.asarray(W_out, dtype=np.float32)
    b_out = np.asarray(b_out, dtype=np.float32)
    # mask is the causal tril mask (per problem spec); causality is
    # implemented structurally on-device.

    run = _get_runner()

    def pack(wslice):
        # [C, DH] -> [P, KO, DH] with C = ko*P + p
        return np.ascontiguousarray(
            wslice.reshape(KO, P, DH).transpose(1, 0, 2))

    def w_hilo(wslice):
        # hi/lo fp8 split: 32W ~= Wh + (Wl term via xh) with W1 for xl term
        w32 = wslice * WSCALE
        wh = w32.astype(f8)
        wl = (w32 - wh.astype(np.float32)).astype(f8)
        w1 = wslice.astype(f8)
        return (pack(wh.astype(np.float32)).astype(f8),
                pack(w1.astype(np.float32)).astype(f8),
                pack(wl.astype(np.float32)).astype(f8))

    def pack_b(bslice):
        # [DH] -> [P, 2] with idx = s*P + p, x32
        return np.ascontiguousarray(
            (bslice * WSCALE).reshape(2, P).T).astype(np.float32)

    tri = np.triu(np.ones((P, P), dtype=np.float32))
    tri2 = np.ascontiguousarray(
        np.broadcast_to(tri[:, None, :], (P, 2, P))).astype(bf)
    ident = np.eye(P, dtype=np.float32).astype(bf)

    xhs, xls = [], []
    for b in range(B):
        xt = np.ascontiguousarray(x[b].T)
        xh = xt.astype(f8)
        xl = ((xt - xh.astype(np.float32)) * WSCALE).astype(f8)
        xhs.append(xh)
        xls.append(xl)

    in_maps = []
    for core in range(N_CORES):
        b, g = divmod(core, GROUPS)
        cs = slice(g * DH, (g + 1) * DH)
        wq3 = w_hilo(W_qkv[:, cs])
        wk3 = w_hilo(W_qkv[:, C:][:, cs])
        wv3 = w_hilo(W_qkv[:, 2 * C:][:, cs])
        in_maps.append({
            "xh": xhs[b], "xl": xls[b],
            "wqh": wq3[0], "wq1": wq3[1], "wql": wq3[2],
            "wkh": wk3[0], "wk1": wk3[1], "wkl": wk3[2],
            "wvh": wv3[0], "wv1": wv3[1], "wvl": wv3[2],
            "wo": np.ascontiguousarray(
                W_out[cs, :].reshape(2, P, C).transpose(1, 0, 2)).astype(bf),
            "bq": pack_b(b_qkv[cs]),
            "bk": pack_b(b_qkv[C:][cs]),
            "bv": np.ascontiguousarray(
                (b_qkv[2 * C:][cs] * WSCALE)[None, :]).astype(np.float32),
            "tri": tri2,
            "ident": ident,
        })

    results = run(in_maps)

    out = np.zeros((B, T, C), dtype=np.float32)
    for core in range(N_CORES):
        b = core // GROUPS
        out[b] += results[core]["out"].astype(np.float32)
    out += b_out[None, None, :]
    return out


# revision 19
# speedup vs baseline: 1.0559x; 1.0156x over previous
"""Trainium2 Bass kernel for nn_CausalSelfAttention (B=2, T=2048, C=1024, 16 heads).

Sharding: 8 cores = 2 batches x 4 head-groups (4 heads each).

v3 design (cost-model driven):
  - QKV projections run as fp8(e4m3) DoubleRow matmuls with hi/lo error
    compensation: 32*x@W = xh@fp8(32W) + fp8(32(x-xh))@fp8(W)
    + xh@fp8(32W - fp8(32W)), all three terms accumulated in one PSUM
    group. DR charges 0.5 cycles/output-col and packs 2 contraction
    chunks per instruction, so this costs 0.75x of bf16 at ~0.3% error
    (plain fp8 would be 0.25x cost but ~5% error -- softmax averaging
    shrinks signal and noise equally, so that error hits the output
    full-strength and blows the 2e-2 gate).
  - S = Q^T K, P (exp output), P@V, and the out-projection stay bf16.
  - exp runs on ACT (~73us busy); PE (~90us busy) is the bottleneck, the
    emission weaves S tiles and filler work by a cost ledger.
  - diagonal causal masking: post-exp multiply by triu-ones on the
    (otherwise idle) gpsimd/Pool engine.
  - O is normalized in [q,d] layout then transposed via PE (identity
    matmul) + DVE copy -- no DMA XBAR on the critical path.
  - out-projection in bf16, partial [T,C] per core; host sums the 4
    tensor-parallel partials per batch and adds b_out.
"""
import sys

if '/opt/trn_rl_repo' not in sys.path:
    sys.path.insert(0, '/opt/trn_rl_repo')

import numpy as np
import ml_dtypes

B, T, C = 2, 2048, 1024
N_HEAD = 16
D = 64
P = 128
N_CORES = 8
GROUPS = N_CORES // B            # 4 tensor-parallel groups per batch
HPC = N_HEAD // GROUPS           # 4 heads per core
DH = HPC * D                     # 256 head dims per core
KO = C // P                      # 8 contraction subtiles for projections
NQB = T // 512                   # 4 q blocks of 512
WSCALE = 32.0                    # fp8 range scaling of W_qkv
SCALE = 1.0 / (np.sqrt(D) * WSCALE * WSCALE)   # exp scale

_CACHE = {}

# weave pacing costs (ns)
PE_CYC = 0.4167


def _build():
    import concourse.mybir as mybir
    import concourse.tile as tile
    from concourse import bacc

    f32 = mybir.dt.float32
    bf16 = mybir.dt.bfloat16
    f16 = mybir.dt.float16
    fp8 = mybir.dt.float8e4
    DR = mybir.MatmulPerfMode.DoubleRow
    EXP = mybir.ActivationFunctionType.Exp
    MUL = mybir.AluOpType.mult
    ADD = mybir.AluOpType.add

    nc = bacc.Bacc("TRN2", target_bir_lowering=False, debug=False,
                   num_devices=N_CORES)

    xh_d = nc.dram_tensor("xh", [C, T], fp8, kind="ExternalInput")
    xl_d = nc.dram_tensor("xl", [C, T], fp8, kind="ExternalInput")
    w_ds = {}
    for nm in ("wqh", "wq1", "wql", "wkh", "wk1", "wkl",
               "wvh", "wv1", "wvl"):
        w_ds[nm] = nc.dram_tensor(nm, [P, KO, DH], fp8, kind="ExternalInput")
    wo_d = nc.dram_tensor("wo", [P, 2, C], bf16, kind="ExternalInput")
    bq_d = nc.dram_tensor("bq", [P, 2], f32, kind="ExternalInput")
    bk_d = nc.dram_tensor("bk", [P, 2], f32, kind="ExternalInput")
    bv_d = nc.dram_tensor("bv", [1, DH], f32, kind="ExternalInput")
    tri_d = nc.dram_tensor("tri", [P, 2, P], bf16, kind="ExternalInput")
    id_d = nc.dram_tensor("ident", [P, P], bf16, kind="ExternalInput")
    out_d = nc.dram_tensor("out", [T, C], f16, kind="ExternalOutput")

    pls = [slice(0, D), slice(D, 2 * D)]

    with tile.TileContext(nc) as tc:
        with (
            tc.tile_pool(name="pp", bufs=1) as pp,
            tc.tile_pool(name="wS", bufs=4) as wS,
            tc.tile_pool(name="wO", bufs=4) as wO,
            tc.tile_pool(name="psS", bufs=2, space="PSUM") as psS,
            tc.tile_pool(name="psU", bufs=4, space="PSUM") as psU,
        ):
            xhs = [pp.tile([P, KO, 512], fp8, tag=f"xh{q}", name=f"xh{q}")
                   for q in range(4)]
            xls = [pp.tile([P, KO, 512], fp8, tag=f"xl{q}", name=f"xl{q}")
                   for q in range(4)]
            wt = {nm: pp.tile([P, KO, DH], fp8, tag=nm, name=nm)
                  for nm in ("wqh", "wq1", "wql", "wkh", "wk1", "wkl",
                             "wvh", "wv1", "wvl")}
            wos = pp.tile([P, 2, C], bf16, tag="wos")
            qts = [[pp.tile([P, 512], bf16, tag=f"qt{s}_{q}", name=f"qt{s}_{q}")
                    for q in range(4)] for s in range(2)]
            kts = [[pp.tile([P, 512], bf16, tag=f"kt{s}_{q}", name=f"kt{s}_{q}")
                    for q in range(4)] for s in range(2)]
            # vo: [k-part, kt-slot, head(2hs+u), d + ones-col]
            vo = pp.tile([P, 16, HPC, D + 1], bf16, tag="vo")
            # pt: per hs P^T tiles [k-part, kt-slot, u, q-col of current block]
            pts = [pp.tile([P, 16, 2, 512], bf16, tag=f"pt{hs}", name=f"pt{hs}")
                   for hs in range(2)]
            ots = [[pp.tile([P, 512], bf16, tag=f"ot{j}_{hs}",
                            name=f"ot{j}_{hs}") for hs in range(2)]
                   for j in range(NQB)]
            bqs = pp.tile([P, 2], f32, tag="bqs")
            bks = pp.tile([P, 2], f32, tag="bks")
            bvrow = pp.tile([1, DH], f32, tag="bvrow")
            bvb = pp.tile([P, DH], f32, tag="bvb")
            trib = pp.tile([P, 2, P], bf16, tag="trib")
            ident = pp.tile([P, P], bf16, tag="ident")
            scr = pp.tile([1, 1], f32, tag="scr")

            # trigger the exp ACT-table load immediately (scratch memset)
            nc.vector.memset(scr[:], 0.0)
            nc.scalar.activation(scr[0:1, 0:1], scr[0:1, 0:1], EXP)
            # ones-column (=WSCALE) of V for softmax denominators
            nc.vector.memset(vo[:, :, :, D:D + 1], WSCALE)

            # ---- loads (sync=SP queue, scalar=ACT queue, gpsimd=SWDGE) ----
            xh_r = xh_d.rearrange("(ko p) t -> p ko t", p=P)
            xl_r = xl_d.rearrange("(ko p) t -> p ko t", p=P)
            for nm in ("wqh", "wq1", "wql"):
                nc.sync.dma_start(wt[nm][:], w_ds[nm][:])
            nc.scalar.dma_start(xhs[0][:], xh_r[:, :, 0:512])
            nc.scalar.dma_start(xls[0][:], xl_r[:, :, 0:512])
            for nm in ("wkh", "wk1", "wkl"):
                nc.gpsimd.dma_start(wt[nm][:], w_ds[nm][:])
            nc.sync.dma_start(bqs[:], bq_d[:])
            nc.sync.dma_start(bks[:], bk_d[:])
            nc.sync.dma_start(trib[:], tri_d[:])
            nc.sync.dma_start(ident[:], id_d[:])
            nc.sync.dma_start(bvrow[:], bv_d[:])
            for nm in ("wvh", "wv1", "wvl"):
                nc.sync.dma_start(wt[nm][:], w_ds[nm][:])
            nc.scalar.dma_start(xhs[1][:], xh_r[:, :, 512:1024])
            nc.scalar.dma_start(xls[1][:], xl_r[:, :, 512:1024])
            nc.scalar.dma_start(xhs[2][:], xh_r[:, :, 1024:1536])
            nc.scalar.dma_start(xls[2][:], xl_r[:, :, 1024:1536])
            nc.scalar.dma_start(xhs[3][:], xh_r[:, :, 1536:2048])
            nc.scalar.dma_start(xls[3][:], xl_r[:, :, 1536:2048])
            nc.sync.dma_start(wos[:], wo_d[:])

            nc.gpsimd.partition_broadcast(bvb[:, :], bvrow[0:1, :])

            # ---------- item constructors (thunk, pe_ns, act_ns) ----------
            # hi/lo fp8 compensation: 32xW = xh@Wh + xl@W1 + xh@Wl, all
            # accumulated in one PSUM group (12 DR steps).
            def qk8_item(q, wpfx, dsts, bias, s_):
                terms = [(wt[wpfx + "h"], xhs), (wt[wpfx + "1"], xls),
                         (wt[wpfx + "l"], xhs)]

                def it():
                    pq = psU.tile([P, 512], f32, tag="ps1",
                                  name=f"pq{q}_{wpfx}_{s_}")
                    for ti, (w8, xsrc) in enumerate(terms):
                        for t in range(KO // 2):
                            nc.tensor.matmul(
                                pq[:],
                                w8[:, 2 * t:2 * t + 2, s_ * P:(s_ + 1) * P],
                                xsrc[q][:, 2 * t:2 * t + 2, :],
                                start=(ti == 0 and t == 0),
                                stop=(ti == 2 and t == KO // 2 - 1),
                                perf_mode=DR)
                    nc.vector.tensor_scalar_add(
                        dsts[s_][q][:], pq[:], bias[:, s_:s_ + 1])
                return (it, 1290, 0)

            def qk8_items(q):
                # order: Q s0, K s0, Q s1, K s1 (heads-split 0 first so the
                # next phase's S(q,0) stream unblocks earliest)
                return [qk8_item(q, "wq", qts, bqs, 0),
                        qk8_item(q, "wk", kts, bks, 0),
                        qk8_item(q, "wq", qts, bqs, 1),
                        qk8_item(q, "wk", kts, bks, 1)]

            def v8_item(q, it_):
                terms = [(wt["wvh"], xhs), (wt["wv1"], xls),
                         (wt["wvl"], xhs)]

                def it():
                    pv = psU.tile([P, DH], f32, tag="ps1",
                                  name=f"pv{q}_{it_}")
                    for ti, (w8, xsrc) in enumerate(terms):
                        for t in range(KO // 2):
                            nc.tensor.matmul(
                                pv[:],
                                xsrc[q][:, 2 * t:2 * t + 2,
                                        it_ * P:(it_ + 1) * P],
                                w8[:, 2 * t:2 * t + 2, :],
                                start=(ti == 0 and t == 0),
                                stop=(ti == 2 and t == KO // 2 - 1),
                                perf_mode=DR)
                    nc.vector.tensor_tensor(
                        vo[:, 4 * q + it_, :, 0:D],
                        pv[:].rearrange("p (h d) -> p h d", h=HPC),
                        bvb.rearrange("p (h d) -> p h d", h=HPC),
                        ADD)
                return (it, 645, 0)

            def s_item(q, hs, i):
                off = max(0, P * i - 512 * q)
                diag = P * i >= 512 * q

                def it():
                    sp = psS.tile([P, 2, 512], f32, tag="sp",
                                  name=f"sp{q}_{hs}_{i}")
                    for u in range(2):
                        nc.tensor.matmul(
                            sp[:, u, off:512],
                            kts[hs][i // 4][pls[u],
                                            (i % 4) * P:(i % 4 + 1) * P],
                            qts[hs][q][pls[u], off:512],
                            start=True, stop=True)
                    nc.scalar.activation(pts[hs][:, i, :, off:512],
                                         sp[:, :, off:512],
                                         EXP, scale=float(SCALE))
                    if diag:
                        nc.gpsimd.tensor_tensor(
                            pts[hs][:, i, :, off:off + P],
                            pts[hs][:, i, :, off:off + P],
                            trib[:], MUL)
                w = 512 - off
                return (it, 2 * w * PE_CYC + 20, 2 * w * 0.833 + 190)

            def pv_item(q, hs, qc, u):
                qt = 4 * q + qc

                def it():
                    po = psU.tile([P, 512], f32, tag="ps1",
                                  name=f"po{q}_{hs}_{qc}_{u}")
                    for kt in range(qt + 1):
                        nc.tensor.matmul(
                            po[:, 0:D + 1],
                            pts[hs][:, kt, u, qc * P:(qc + 1) * P],
                            vo[:, kt, 2 * hs + u, :],
                            start=(kt == 0), stop=(kt == qt))
                    rd = wS.tile([P, 1], f32, tag="rd",
                                 name=f"rd{q}_{hs}_{qc}_{u}")
                    if u == 0:
                        osbs[(q, hs, qc)] = wS.tile(
                            [P, 2, D], bf16, tag="osb",
                            name=f"osb{q}_{hs}_{qc}")
                    osb = osbs[(q, hs, qc)]
                    nc.vector.reciprocal_approx_fast(rd[:], po[:, D:D + 1])
                    nc.vector.tensor_scalar_mul(
                        osb[:, u, :], po[:, 0:D], rd[:])
                return (it, (qt + 1) * 27 + 30, 0)

            osbs = {}

            def tr_item(q, hs, qc):
                def it():
                    ptr = psU.tile([P, P], bf16, tag="ps1",
                                   name=f"ptr{q}_{hs}_{qc}")
                    nc.tensor.transpose(
                        ptr[:], osbs[(q, hs, qc)].rearrange("p u d -> p (u d)"),
                        ident[:])
                    nc.vector.tensor_copy(
                        ots[q][hs][:, qc * P:(qc + 1) * P], ptr[:])
                return (it, 75, 0)

            def c_item(jj, mo, n):
                def it():
                    pc = psU.tile([P, 512], f32, tag="ps1",
                                  name=f"pc{jj}_{mo}_{n}")
                    for s in range(2):
                        nc.tensor.matmul(
                            pc[:],
                            ots[jj][s][:, mo * P:(mo + 1) * P],
                            wos[:, s, n * 512:(n + 1) * 512],
                            start=(s == 0), stop=(s == 1))
                    if n == 0:
                        obs[(jj, mo)] = wO.tile([P, C], f16, tag="ob",
                                                name=f"ob{jj}_{mo}")
                    ob = obs[(jj, mo)]
                    nc.vector.tensor_copy(
                        ob[:, n * 512:(n + 1) * 512], pc[:])
                    if n == 1:
                        m = 4 * jj + mo
                        nc.gpsimd.dma_start(
                            out_d[m * P:(m + 1) * P, :], ob[:])
                return (it, 430, 0)

            obs = {}

            # ---------- schedule ----------
            # master stream: all S tiles in (q, hs, i) order; ACT must never
            # starve, PE (the bottleneck) must never park behind a stalled
            # S matmul. Fillers are paced by a global PE-vs-ACT cost ledger;
            # per-phase filler assignment matches each segment's PE deficit
            # (~611ns per S tile).
            ledger = {"act": 0.0, "pe": 0.0}

            def weave(masters, fillers, extra=()):
                fi = 0
                for k, (it, pe, act) in enumerate(masters):
                    it()
                    ledger["act"] += act
                    ledger["pe"] += pe
                    while fi < len(fillers) and \
                            ledger["pe"] + 400 < ledger["act"]:
                        f, fpe, _ = fillers[fi]
                        f()
                        ledger["pe"] += fpe
                        fi += 1
                    for pos, item in extra:
                        if pos == k:
                            item[0]()
                            ledger["pe"] += item[1]
                while fi < len(fillers):
                    f, fpe, _ = fillers[fi]
                    f()
                    ledger["pe"] += fpe
                    fi += 1

            def pv_pair(q, hs, qc):
                return [pv_item(q, hs, qc, 0), pv_item(q, hs, qc, 1)]

            # prologue: q=0 projections at term granularity so PE starts as
            # soon as the first weight chunk lands
            def qk_term(q, wpfx, s_, ti, dsts=None, bias=None):
                nms = (wpfx + "h", wpfx + "1", wpfx + "l")
                xsrcs = (xhs, xls, xhs)
                key = ("pro", q, wpfx, s_)

                def it():
                    if ti == 0:
                        pro_ps[key] = psU.tile([P, 512], f32, tag="ps1",
                                               name=f"pq{q}_{wpfx}_{s_}")
                    pq = pro_ps[key]
                    w8 = wt[nms[ti]]
                    for t in range(KO // 2):
                        nc.tensor.matmul(
                            pq[:],
                            w8[:, 2 * t:2 * t + 2, s_ * P:(s_ + 1) * P],
                            xsrcs[ti][q][:, 2 * t:2 * t + 2, :],
                            start=(ti == 0 and t == 0),
                            stop=(ti == 2 and t == KO // 2 - 1),
                            perf_mode=DR)
                    if ti == 2:
                        nc.vector.tensor_scalar_add(
                            dsts[s_][q][:], pq[:], bias[:, s_:s_ + 1])
                return (it, 430, 0)

            pro_ps = {}
            for ti in range(3):
                qk_term(0, "wq", 0, ti, qts, bqs)[0]()
                qk_term(0, "wk", 0, ti, kts, bks)[0]()

            masters0 = [s_item(0, 0, i) for i in range(4)]
            fillers0 = ([qk8_item(0, "wq", qts, bqs, 1),
                         qk8_item(0, "wk", kts, bks, 1)]
                        + [v8_item(0, k) for k in range(4)])
            weave(masters0, fillers0)

            # per-phase filler assignment (see header comment)
            FA = {1: [], 2: [], 3: []}
            FB = {0: [], 1: [], 2: [], 3: []}
            # phase 0 B: PV(0,0), tr, A(1), v8(1)
            for qc in range(4):
                FB[0] += pv_pair(0, 0, qc)
                if qc >= 1:
                    FB[0].append(tr_item(0, 0, qc - 1))
            FB[0].append(tr_item(0, 0, 3))
            FB[0] += qk8_items(1)
            FB[0] += [v8_item(1, k) for k in range(4)]
            # phase 1 A: PV(0,1), tr, A(2) s0-half, c(0)[0:2]
            for qc in range(4):
                FA[1] += pv_pair(0, 1, qc)
                if qc >= 1:
                    FA[1].append(tr_item(0, 1, qc - 1))
            FA[1].append(tr_item(0, 1, 3))
            FA[1] += qk8_items(2)[0:2]
            FA[1] += [c_item(0, 0, 0), c_item(0, 0, 1)]
            # phase 1 B: PV(1,0), tr, A(2) s1-half, v8(2)
            for qc in range(4):
                FB[1] += pv_pair(1, 0, qc)
                if qc >= 1:
                    FB[1].append(tr_item(1, 0, qc - 1))
            FB[1].append(tr_item(1, 0, 3))
            FB[1] += qk8_items(2)[2:4]
            FB[1] += [v8_item(2, k) for k in range(4)]
            # phase 2 A: PV(1,1), tr, c(0)[rest], v8(3)
            for qc in range(4):
                FA[2] += pv_pair(1, 1, qc)
                if qc >= 1:
                    FA[2].append(tr_item(1, 1, qc - 1))
            FA[2].append(tr_item(1, 1, 3))
            for mo in range(1, 4):
                FA[2] += [c_item(0, mo, 0), c_item(0, mo, 1)]
            FA[2] += [v8_item(3, k) for k in range(4)]
            # phase 2 B: PV(2,0), tr, A(3) all
            for qc in range(4):
                FB[2] += pv_pair(2, 0, qc)
                if qc >= 1:
                    FB[2].append(tr_item(2, 0, qc - 1))
            FB[2].append(tr_item(2, 0, 3))
            FB[2] += qk8_items(3)
            # phase 3 A: PV(2,1), tr, c(1) all, c(2)[0:3]
            for qc in range(4):
                FA[3] += pv_pair(2, 1, qc)
                if qc >= 1:
                    FA[3].append(tr_item(2, 1, qc - 1))
            FA[3].append(tr_item(2, 1, 3))
            for mo in range(4):
                FA[3] += [c_item(1, mo, 0), c_item(1, mo, 1)]
            FA[3] += [c_item(2, 0, 0), c_item(2, 0, 1), c_item(2, 1, 0)]
            # phase 3 B: PV(3,0), tr, c(2)[rest]; PV(3,1,0..2) placed late
            # via `extra` (they need the last exps of this segment)
            for qc in range(4):
                FB[3] += pv_pair(3, 0, qc)
                if qc >= 1:
                    FB[3].append(tr_item(3, 0, qc - 1))
            FB[3].append(tr_item(3, 0, 3))
            FB[3] += [c_item(2, 1, 1), c_item(2, 2, 0), c_item(2, 2, 1),
                      c_item(2, 3, 0), c_item(2, 3, 1)]

            for q in range(4):
                if q > 0:
                    mastersA = [s_item(q, 0, i) for i in range(4 * q + 4)]
                    weave(mastersA, FA[q])
                mastersB = [s_item(q, 1, i) for i in range(4 * q + 4)]
                if q == 3:
                    extra = [(14, pv_item(3, 1, 0, 0)),
                             (14, pv_item(3, 1, 0, 1)),
                             (15, pv_item(3, 1, 1, 0)),
                             (15, pv_item(3, 1, 1, 1)),
                             (15, tr_item(3, 1, 0))]
                    weave(mastersB, FB[q], extra)
                else:
                    weave(mastersB, FB[q])

            # tail: remaining PV(3,1), transposes, c(3,*) staggered
            pv_item(3, 1, 2, 0)[0]()
            pv_item(3, 1, 2, 1)[0]()
            tr_item(3, 1, 1)[0]()
            c_item(3, 0, 0)[0]()
            c_item(3, 0, 1)[0]()
            pv_item(3, 1, 3, 0)[0]()
            pv_item(3, 1, 3, 1)[0]()
            tr_item(3, 1, 2)[0]()
            c_item(3, 1, 0)[0]()
            c_item(3, 1, 1)[0]()
            tr_item(3, 1, 3)[0]()
            c_item(3, 2, 0)[0]()
            c_item(3, 2, 1)[0]()
            c_item(3, 3, 0)[0]()
            c_item(3, 3, 1)[0]()

    nc.compile()
    return nc


def _get_nc():
    if "nc" not in _CACHE:
        _CACHE["nc"] = _build()
    return _CACHE["nc"]


def _get_runner():
    """Build the jitted SPMD executor once (mirrors bass2jax.run_bass_via_pjrt
    but caches the jitted function so repeat calls skip retrace/recompile)."""
    if "runner" in _CACHE:
        return _CACHE["runner"]
    import jax
    import numpy as _np
    from jax.sharding import Mesh, PartitionSpec
    from jax.experimental.shard_map import shard_map
    import concourse.mybir as mybir
    from concourse import bass2jax

    nc = _get_nc()
    bass2jax.install_neuronx_cc_hook()

    partition_name = (nc.partition_id_tensor.name
                      if nc.partition_id_tensor else None)
    in_names, out_names, out_avals, zero_shapes = [], [], [], []
    for alloc in nc.m.functions[0].allocations:
        if not isinstance(alloc, mybir.MemoryLocationSet):
            continue
        name = alloc.memorylocations[0].name
        if alloc.kind == "ExternalInput":
            if name != partition_name:
                in_names.append(name)
        elif alloc.kind == "ExternalOutput":
            out_avals.append(jax.core.ShapedArray(
                tuple(alloc.tensor_shape), mybir.dt.np(alloc.dtype)))
            out_names.append(name)
            zero_shapes.append((tuple(alloc.tensor_shape),
                                mybir.dt.np(alloc.dtype)))
    n_params = len(in_names)
    n_outs = len(out_names)
    all_names = in_names + out_names
    if partition_name is not None:
        all_names = all_names + [partition_name]

    def _body(*args):
        operands = list(args)
        if partition_name is not None:
            operands.append(bass2jax.partition_id_tensor())
        outs = bass2jax._bass_exec_p.bind(
            *operands,
            out_avals=tuple(out_avals),
            in_names=tuple(all_names),
            out_names=tuple(out_names),
            lowering_input_output_aliases=(),
            sim_require_finite=True,
            sim_require_nnan=True,
            nc=nc,
        )
        return tuple(outs)

    devices = jax.devices()[:N_CORES]
    mesh = Mesh(_np.asarray(devices), ("core",))
    donate = tuple(range(n_params, n_params + n_outs))
    sharded = jax.jit(
        shard_map(_body, mesh=mesh,
                  in_specs=(PartitionSpec("core"),) * (n_params + n_outs),
                  out_specs=(PartitionSpec("core"),) * n_outs,
                  check_rep=False),
        donate_argnums=donate, keep_unused=True)

    def run(in_maps):
        concat_in = [
            _np.concatenate([_np.asarray(m[name]) for m in in_maps], axis=0)
            for name in in_names]
        concat_zeros = [
            _np.zeros((N_CORES * sh[0], *sh[1:]), dtype)
            for sh, dtype in zero_shapes]
        out_arrs = sharded(*concat_in, *concat_zeros)
        return [
            {name: _np.asarray(out_arrs[i]).reshape(
                N_CORES, *zero_shapes[i][0])[c]
             for i, name in enumerate(out_names)}
            for c in range(N_CORES)]

    _CACHE["runner"] = run
    return run


def _fp8():
    return (ml_dtypes.float8_e4m3fn if hasattr(ml_dtypes, 'float8_e4m3fn')
            else ml_dtypes.float8_e4m3)


def kernel(x, mask, W_qkv, b_qkv, W_out, b_out):
    bf = ml_dtypes.bfloat16
    f8 = _fp8()
    x = np.asarray(x, dtype=np.float32)
    W_qkv = np.asarray(W_qkv, dtype=np.float32)
    b_qkv = np.asarray(b_qkv, dtype=np.float32)
    W_out = np.asarray(W_out, dtype=np.float32)
    b_out = np.asarray(b_out, dtype=np.float32)
    # mask is the causal tril mask (per problem spec); causality is
    # implemented structurally on-device.

    run = _get_runner()

    def pack(wslice):
        # [C, DH] -> [P, KO, DH] with C = ko*P + p
        return np.ascontiguousarray(
            wslice.reshape(KO, P, DH).transpose(1, 0, 2))

    def w_hilo(wslice):
        # hi/lo fp8 split: 32W ~= Wh + (Wl term via xh) with W1 for xl term
        w32 = wslice * WSCALE
        wh = w32.astype(f8)
        wl = (w32 - wh.astype(np.float32)).astype(f8)
        w1 = wslice.astype(f8)
        return (pack(wh.astype(np.float32)).astype(f8),
                pack(w1.astype(np.float32)).astype(f8),
                pack(wl.astype(np.float32)).astype(f8))

    def pack_b(bslice):
        # [DH] -> [P, 2] with idx = s*P + p, x32
        return np.ascontiguousarray(
            (bslice * WSCALE).reshape(2, P).T).astype(np.float32)

    tri = np.triu(np.ones((P, P), dtype=np.float32))
    tri2 = np.ascontiguousarray(
        np.broadcast_to(tri[:, None, :], (P, 2, P))).astype(bf)
    ident = np.eye(P, dtype=np.float32).astype(bf)

    xhs, xls = [], []
    for b in range(B):
        xt = np.ascontiguousarray(x[b].T)
        xh = xt.astype(f8)
        xl = ((xt - xh.astype(np.float32)) * WSCALE).astype(f8)
        xhs.append(xh)
        xls.append(xl)

    in_maps = []
    for core in range(N_CORES):
        b, g = divmod(core, GROUPS)
        cs = slice(g * DH, (g + 1) * DH)
        wq3 = w_hilo(W_qkv[:, cs])
        wk3 = w_hilo(W_qkv[:, C:][:, cs])
        wv3 = w_hilo(W_qkv[:, 2 * C:][:, cs])
        in_maps.append({
            "xh": xhs[b], "xl": xls[b],
            "wqh": wq3[0], "wq1": wq3[1], "wql": wq3[2],
            "wkh": wk3[0], "wk1": wk3[1], "wkl": wk3[2],
            "wvh": wv3[0], "wv1": wv3[1], "wvl": wv3[2],
            "wo": np.ascontiguousarray(
                W_out[cs, :].reshape(2, P, C).transpose(1, 0, 2)).astype(bf),
            "bq": pack_b(b_qkv[cs]),
            "bk": pack_b(b_qkv[C:][cs]),
            "bv": np.ascontiguousarray(
                (b_qkv[2 * C:][cs] * WSCALE)[None, :]).astype(np.float32),
            "tri": tri2,
            "ident": ident,
        })

    results = run(in_maps)

    out = np.zeros((B, T, C), dtype=np.float32)
    for core in range(N_CORES):
        b = core // GROUPS
        out[b] += results[core]["out"].astype(np.float32)
    out += b_out[None, None, :]
    return out


# revision 23
# speedup vs baseline: 1.1266x; 1.0670x over previous
"""Trainium2 Bass kernel for nn_CausalSelfAttention (B=2, T=2048, C=1024, 16 heads).

Sharding: 8 cores = 2 batches x 4 head-groups (4 heads each).

v3 design (cost-model driven):
  - QKV projections run as fp8(e4m3) DoubleRow matmuls with hi/lo error
    compensation: 32*x@W = xh@fp8(32W) + fp8(32(x-xh))@fp8(W)
    + xh@fp8(32W - fp8(32W)), all three terms accumulated in one PSUM
    group. DR charges 0.5 cycles/output-col and packs 2 contraction
    chunks per instruction, so this costs 0.75x of bf16 at ~0.3% error
    (plain fp8 would be 0.25x cost but ~5% error -- softmax averaging
    shrinks signal and noise equally, so that error hits the output
    full-strength and blows the 2e-2 gate).
  - S = Q^T K, P (exp output), P@V, and the out-projection stay bf16.
  - exp runs on ACT (~73us busy); PE (~90us busy) is the bottleneck, the
    emission weaves S tiles and filler work by a cost ledger.
  - diagonal causal masking: post-exp multiply by triu-ones on the
    (otherwise idle) gpsimd/Pool engine.
  - O is normalized in [q,d] layout then transposed via PE (identity
    matmul) + DVE copy -- no DMA XBAR on the critical path.
  - out-projection in bf16, partial [T,C] per core; host sums the 4
    tensor-parallel partials per batch and adds b_out.
"""
import sys

if '/opt/trn_rl_repo' not in sys.path:
    sys.path.insert(0, '/opt/trn_rl_repo')

import numpy as np
import ml_dtypes

B, T, C = 2, 2048, 1024
N_HEAD = 16
D = 64
P = 128
N_CORES = 8
GROUPS = N_CORES // B            # 4 tensor-parallel groups per batch
HPC = N_HEAD // GROUPS           # 4 heads per core
DH = HPC * D                     # 256 head dims per core
KO = C // P                      # 8 contraction subtiles for projections
NQB = T // 512                   # 4 q blocks of 512
WSCALE = 32.0                    # fp8 range scaling of W_qkv
SCALE = 1.0 / (np.sqrt(D) * WSCALE * WSCALE)   # exp scale

_CACHE = {}

# weave pacing costs (ns)
PE_CYC = 0.4167


def _build():
    import concourse.mybir as mybir
    import concourse.tile as tile
    from concourse import bacc

    f32 = mybir.dt.float32
    bf16 = mybir.dt.bfloat16
    f16 = mybir.dt.float16
    fp8 = mybir.dt.float8e4
    DR = mybir.MatmulPerfMode.DoubleRow
    EXP = mybir.ActivationFunctionType.Exp
    MUL = mybir.AluOpType.mult
    ADD = mybir.AluOpType.add

    nc = bacc.Bacc("TRN2", target_bir_lowering=False, debug=False,
                   num_devices=N_CORES)

    xh_d = nc.dram_tensor("xh", [C, T], fp8, kind="ExternalInput")
    xl_d = nc.dram_tensor("xl", [C, T], fp8, kind="ExternalInput")
    w_ds = {}
    for nm in ("wqh", "wq1", "wql", "wkh", "wk1", "wkl",
               "wvh", "wv1", "wvl"):
        w_ds[nm] = nc.dram_tensor(nm, [P, KO, DH], fp8, kind="ExternalInput")
    wo_d = nc.dram_tensor("wo", [P, 2, C], bf16, kind="ExternalInput")
    bq_d = nc.dram_tensor("bq", [P, 2], f32, kind="ExternalInput")
    bk_d = nc.dram_tensor("bk", [P, 2], f32, kind="ExternalInput")
    bv_d = nc.dram_tensor("bv", [1, DH], f32, kind="ExternalInput")
    tri_d = nc.dram_tensor("tri", [P, 2, P], bf16, kind="ExternalInput")
    id_d = nc.dram_tensor("ident", [P, P], bf16, kind="ExternalInput")
    out_d = nc.dram_tensor("out", [T, C], f16, kind="ExternalOutput")

    pls = [slice(0, D), slice(D, 2 * D)]

    with tile.TileContext(nc) as tc:
        with (
            tc.tile_pool(name="pp", bufs=1) as pp,
            tc.tile_pool(name="wS", bufs=4) as wS,
            tc.tile_pool(name="wO", bufs=4) as wO,
            tc.tile_pool(name="psS", bufs=2, space="PSUM") as psS,
            tc.tile_pool(name="psU", bufs=4, space="PSUM") as psU,
        ):
            xhs = [pp.tile([P, KO, 512], fp8, tag=f"xh{q}", name=f"xh{q}")
                   for q in range(4)]
            xls = [pp.tile([P, KO, 512], fp8, tag=f"xl{q}", name=f"xl{q}")
                   for q in range(4)]
            wt = {nm: pp.tile([P, KO, DH], fp8, tag=nm, name=nm)
                  for nm in ("wqh", "wq1", "wql", "wkh", "wk1", "wkl",
                             "wvh", "wv1", "wvl")}
            wos = pp.tile([P, 2, C], bf16, tag="wos")
            qts = [[pp.tile([P, 512], bf16, tag=f"qt{s}_{q}", name=f"qt{s}_{q}")
                    for q in range(4)] for s in range(2)]
            kts = [[pp.tile([P, 512], bf16, tag=f"kt{s}_{q}", name=f"kt{s}_{q}")
                    for q in range(4)] for s in range(2)]
            # vo: [k-part, kt-slot, head(2hs+u), d + ones-col]
            vo = pp.tile([P, 16, HPC, D + 1], bf16, tag="vo")
            # pt: per hs P^T tiles [k-part, kt-slot, u, q-col of current block]
            pts = [pp.tile([P, 16, 2, 512], bf16, tag=f"pt{hs}", name=f"pt{hs}")
                   for hs in range(2)]
            ots = [[pp.tile([P, 512], bf16, tag=f"ot{j}_{hs}",
                            name=f"ot{j}_{hs}") for hs in range(2)]
                   for j in range(NQB)]
            bqs = pp.tile([P, 2], f32, tag="bqs")
            bks = pp.tile([P, 2], f32, tag="bks")
            bvrow = pp.tile([1, DH], f32, tag="bvrow")
            bvb = pp.tile([P, DH], f32, tag="bvb")
            trib = pp.tile([P, 2, P], bf16, tag="trib")
            ident = pp.tile([P, P], bf16, tag="ident")
            scr = pp.tile([1, 1], f32, tag="scr")

            # trigger the exp ACT-table load immediately (scratch memset)
            nc.vector.memset(scr[:], 0.0)
            nc.scalar.activation(scr[0:1, 0:1], scr[0:1, 0:1], EXP)
            # ones-column (=WSCALE) of V for softmax denominators
            nc.vector.memset(vo[:, :, :, D:D + 1], WSCALE)

            # ---- loads (sync=SP queue, scalar=ACT queue, gpsimd=SWDGE) ----
            xh_r = xh_d.rearrange("(ko p) t -> p ko t", p=P)
            xl_r = xl_d.rearrange("(ko p) t -> p ko t", p=P)
            nc.sync.dma_start(wt["wqh"][:], w_ds["wqh"][:])
            nc.scalar.dma_start(xhs[0][:], xh_r[:, :, 0:512])
            nc.sync.dma_start(wt["wq1"][:], w_ds["wq1"][:])
            nc.scalar.dma_start(xls[0][:], xl_r[:, :, 0:512])
            nc.sync.dma_start(wt["wql"][:], w_ds["wql"][:])
            nc.sync.dma_start(bqs[:], bq_d[:])
            for nm in ("wkh", "wk1", "wkl"):
                nc.gpsimd.dma_start(wt[nm][:], w_ds[nm][:])
            nc.sync.dma_start(bks[:], bk_d[:])
            nc.sync.dma_start(trib[:], tri_d[:])
            nc.sync.dma_start(ident[:], id_d[:])
            nc.sync.dma_start(bvrow[:], bv_d[:])
            for nm in ("wvh", "wv1", "wvl"):
                nc.sync.dma_start(wt[nm][:], w_ds[nm][:])
            nc.scalar.dma_start(xhs[1][:], xh_r[:, :, 512:1024])
            nc.scalar.dma_start(xls[1][:], xl_r[:, :, 512:1024])
            nc.scalar.dma_start(xhs[2][:], xh_r[:, :, 1024:1536])
            nc.scalar.dma_start(xls[2][:], xl_r[:, :, 1024:1536])
            nc.scalar.dma_start(xhs[3][:], xh_r[:, :, 1536:2048])
            nc.scalar.dma_start(xls[3][:], xl_r[:, :, 1536:2048])
            nc.sync.dma_start(wos[:], wo_d[:])

            nc.gpsimd.partition_broadcast(bvb[:, :], bvrow[0:1, :])

            # ---------- item constructors (thunk, pe_ns, act_ns) ----------
            # hi/lo fp8 compensation: 32xW = xh@Wh + xl@W1 + xh@Wl, all
            # accumulated in one PSUM group (12 DR steps).
            def qk8_item(q, wpfx, dsts, bias, s_):
                terms = [(wt[wpfx + "h"], xhs), (wt[wpfx + "1"], xls),
                         (wt[wpfx + "l"], xhs)]

                def it():
                    pq = psU.tile([P, 512], f32, tag="ps1",
                                  name=f"pq{q}_{wpfx}_{s_}")
                    for ti, (w8, xsrc) in enumerate(terms):
                        for t in range(KO // 2):
                            nc.tensor.matmul(
                                pq[:],
                                w8[:, 2 * t:2 * t + 2, s_ * P:(s_ + 1) * P],
                                xsrc[q][:, 2 * t:2 * t + 2, :],
                                start=(ti == 0 and t == 0),
                                stop=(ti == 2 and t == KO // 2 - 1),
                                perf_mode=DR)
                    nc.vector.tensor_scalar_add(
                        dsts[s_][q][:], pq[:], bias[:, s_:s_ + 1])
                return (it, 1290, 0)

            def qk8_items(q):
                # order: Q s0, K s0, Q s1, K s1 (heads-split 0 first so the
                # next phase's S(q,0) stream unblocks earliest)
                return [qk8_item(q, "wq", qts, bqs, 0),
                        qk8_item(q, "wk", kts, bks, 0),
                        qk8_item(q, "wq", qts, bqs, 1),
                        qk8_item(q, "wk", kts, bks, 1)]

            def v8_item(q, it_):
                terms = [(wt["wvh"], xhs), (wt["wv1"], xls),
                         (wt["wvl"], xhs)]

                def it():
                    pv = psU.tile([P, DH], f32, tag="ps1",
                                  name=f"pv{q}_{it_}")
                    for ti, (w8, xsrc) in enumerate(terms):
                        for t in range(KO // 2):
                            nc.tensor.matmul(
                                pv[:],
                                xsrc[q][:, 2 * t:2 * t + 2,
                                        it_ * P:(it_ + 1) * P],
                                w8[:, 2 * t:2 * t + 2, :],
                                start=(ti == 0 and t == 0),
                                stop=(ti == 2 and t == KO // 2 - 1),
                                perf_mode=DR)
                    nc.vector.tensor_tensor(
                        vo[:, 4 * q + it_, :, 0:D],
                        pv[:].rearrange("p (h d) -> p h d", h=HPC),
                        bvb.rearrange("p (h d) -> p h d", h=HPC),
                        ADD)
                return (it, 645, 0)

            def s_item(q, hs, i):
                off = max(0, P * i - 512 * q)
                diag = P * i >= 512 * q

                def it():
                    sp = psS.tile([P, 2, 512], f32, tag="sp",
                                  name=f"sp{q}_{hs}_{i}")
                    for u in range(2):
                        nc.tensor.matmul(
                            sp[:, u, off:512],
                            kts[hs][i // 4][pls[u],
                                            (i % 4) * P:(i % 4 + 1) * P],
                            qts[hs][q][pls[u], off:512],
                            start=True, stop=True)
                    nc.scalar.activation(pts[hs][:, i, :, off:512],
                                         sp[:, :, off:512],
                                         EXP, scale=float(SCALE))
                    if diag:
                        nc.gpsimd.tensor_tensor(
                            pts[hs][:, i, :, off:off + P],
                            pts[hs][:, i, :, off:off + P],
                            trib[:], MUL)
                w = 512 - off
                return (it, 2 * w * PE_CYC + 20, 2 * w * 0.833 + 190)

            def pv_item(q, hs, qc, u):
                qt = 4 * q + qc

                def it():
                    po = psU.tile([P, 512], f32, tag="ps1",
                                  name=f"po{q}_{hs}_{qc}_{u}")
                    for kt in range(qt + 1):
                        nc.tensor.matmul(
                            po[:, 0:D + 1],
                            pts[hs][:, kt, u, qc * P:(qc + 1) * P],
                            vo[:, kt, 2 * hs + u, :],
                            start=(kt == 0), stop=(kt == qt))
                    rd = wS.tile([P, 1], f32, tag="rd",
                                 name=f"rd{q}_{hs}_{qc}_{u}")
                    if u == 0:
                        osbs[(q, hs, qc)] = wS.tile(
                            [P, 2, D], bf16, tag="osb",
                            name=f"osb{q}_{hs}_{qc}")
                    osb = osbs[(q, hs, qc)]
                    nc.vector.reciprocal_approx_fast(rd[:], po[:, D:D + 1])
                    nc.vector.tensor_scalar_mul(
                        osb[:, u, :], po[:, 0:D], rd[:])
                return (it, (qt + 1) * 27 + 30, 0)

            osbs = {}

            def tr_item(q, hs, qc):
                def it():
                    ptr = psU.tile([P, P], bf16, tag="ps1",
                                   name=f"ptr{q}_{hs}_{qc}")
                    nc.tensor.transpose(
                        ptr[:], osbs[(q, hs, qc)].rearrange("p u d -> p (u d)"),
                        ident[:])
                    nc.vector.tensor_copy(
                        ots[q][hs][:, qc * P:(qc + 1) * P], ptr[:])
                return (it, 75, 0)

            def c_item(jj, mo, n):
                def it():
                    pc = psU.tile([P, 512], f32, tag="ps1",
                                  name=f"pc{jj}_{mo}_{n}")
                    for s in range(2):
                        nc.tensor.matmul(
                            pc[:],
                            ots[jj][s][:, mo * P:(mo + 1) * P],
                            wos[:, s, n * 512:(n + 1) * 512],
                            start=(s == 0), stop=(s == 1))
                    if n == 0:
                        obs[(jj, mo)] = wO.tile([P, C], f16, tag="ob",
                                                name=f"ob{jj}_{mo}")
                    ob = obs[(jj, mo)]
                    nc.vector.tensor_copy(
                        ob[:, n * 512:(n + 1) * 512], pc[:])
                    if n == 1:
                        m = 4 * jj + mo
                        nc.sync.dma_start(
                            out_d[m * P:(m + 1) * P, :], ob[:])
                return (it, 430, 0)

            obs = {}

            # ---------- schedule ----------
            # master stream: all S tiles in (q, hs, i) order; ACT must never
            # starve, PE (the bottleneck) must never park behind a stalled
            # S matmul. Fillers are paced by a global PE-vs-ACT cost ledger;
            # per-phase filler assignment matches each segment's PE deficit
            # (~611ns per S tile).
            def weave(masters, fillers, extra=()):
                act_t, pe_t, fi = 0.0, 0.0, 0
                for k, (it, pe, act) in enumerate(masters):
                    it()
                    act_t += act
                    pe_t += pe
                    while fi < len(fillers) and pe_t + 400 < act_t:
                        f, fpe, _ = fillers[fi]
                        f()
                        pe_t += fpe
                        fi += 1
                    for pos, item in extra:
                        if pos == k:
                            item[0]()
                            pe_t += item[1]
                while fi < len(fillers):
                    f, fpe, _ = fillers[fi]
                    f()
                    fi += 1

            def pv_pair(q, hs, qc):
                return [pv_item(q, hs, qc, 0), pv_item(q, hs, qc, 1)]

            # prologue: q=0 projections at term granularity so PE starts as
            # soon as the first weight chunk lands
            def qk_term(q, wpfx, s_, ti, dsts=None, bias=None):
                nms = (wpfx + "h", wpfx + "1", wpfx + "l")
                xsrcs = (xhs, xls, xhs)
                key = ("pro", q, wpfx, s_)

                def it():
                    if ti == 0:
                        pro_ps[key] = psU.tile([P, 512], f32, tag="ps1",
                                               name=f"pq{q}_{wpfx}_{s_}")
                    pq = pro_ps[key]
                    w8 = wt[nms[ti]]
                    for t in range(KO // 2):
                        nc.tensor.matmul(
                            pq[:],
                            w8[:, 2 * t:2 * t + 2, s_ * P:(s_ + 1) * P],
                            xsrcs[ti][q][:, 2 * t:2 * t + 2, :],
                            start=(ti == 0 and t == 0),
                            stop=(ti == 2 and t == KO // 2 - 1),
                            perf_mode=DR)
                    if ti == 2:
                        nc.vector.tensor_scalar_add(
                            dsts[s_][q][:], pq[:], bias[:, s_:s_ + 1])
                return (it, 430, 0)

            pro_ps = {}
            for ti in range(3):
                qk_term(0, "wq", 0, ti, qts, bqs)[0]()
                qk_term(0, "wk", 0, ti, kts, bks)[0]()

            masters0 = [s_item(0, 0, i) for i in range(4)]
            fillers0 = ([qk8_item(0, "wq", qts, bqs, 1),
                         qk8_item(0, "wk", kts, bks, 1)]
                        + [v8_item(0, k) for k in range(4)])
            weave(masters0, fillers0)

            # per-phase filler assignment (see header comment)
            FA = {1: [], 2: [], 3: []}
            FB = {0: [], 1: [], 2: [], 3: []}
            # phase 0 B: PV(0,0), tr, A(1), v8(1)
            for qc in range(4):
                FB[0] += pv_pair(0, 0, qc)
                if qc >= 1:
                    FB[0].append(tr_item(0, 0, qc - 1))
            FB[0].append(tr_item(0, 0, 3))
            FB[0] += qk8_items(1)
            FB[0] += [v8_item(1, k) for k in range(4)]
            # phase 1 A: PV(0,1), tr, A(2) s0-half, v8(2)
            for qc in range(4):
                FA[1] += pv_pair(0, 1, qc)
                if qc >= 1:
                    FA[1].append(tr_item(0, 1, qc - 1))
            FA[1].append(tr_item(0, 1, 3))
            FA[1] += qk8_items(2)[0:2]
            FA[1] += [v8_item(2, k) for k in range(4)]
            # phase 1 B: PV(1,0), tr, A(2) s1-half
            for qc in range(4):
                FB[1] += pv_pair(1, 0, qc)
                if qc >= 1:
                    FB[1].append(tr_item(1, 0, qc - 1))
            FB[1].append(tr_item(1, 0, 3))
            FB[1] += qk8_items(2)[2:4]
            # phase 2 A: PV(1,1), tr, A(3) s0-half, v8(3)
            for qc in range(4):
                FA[2] += pv_pair(1, 1, qc)
                if qc >= 1:
                    FA[2].append(tr_item(1, 1, qc - 1))
            FA[2].append(tr_item(1, 1, 3))
            FA[2] += qk8_items(3)[0:2]
            FA[2] += [v8_item(3, k) for k in range(4)]
            # phase 2 B: PV(2,0), tr, A(3) s1-half
            for qc in range(4):
                FB[2] += pv_pair(2, 0, qc)
                if qc >= 1:
                    FB[2].append(tr_item(2, 0, qc - 1))
            FB[2].append(tr_item(2, 0, 3))
            FB[2] += qk8_items(3)[2:4]
            # phase 3 A: PV(2,1), tr, c(1) all, c(2) all
            for qc in range(4):
                FA[3] += pv_pair(2, 1, qc)
                if qc >= 1:
                    FA[3].append(tr_item(2, 1, qc - 1))
            FA[3].append(tr_item(2, 1, 3))
            for mo in range(4):
                FA[3] += [c_item(1, mo, 0), c_item(1, mo, 1)]
            for mo in range(4):
                FA[3] += [c_item(2, mo, 0), c_item(2, mo, 1)]
            # phase 3 B: PV(3,0), tr, c(0) all; PV(3,1,0..2) placed late
            # via `extra` (they need the last exps of this segment)
            for qc in range(4):
                FB[3] += pv_pair(3, 0, qc)
                if qc >= 1:
                    FB[3].append(tr_item(3, 0, qc - 1))
            FB[3].append(tr_item(3, 0, 3))
            for mo in range(4):
                FB[3] += [c_item(0, mo, 0), c_item(0, mo, 1)]

            for q in range(4):
                if q > 0:
                    mastersA = [s_item(q, 0, i) for i in range(4 * q + 4)]
                    weave(mastersA, FA[q])
                mastersB = [s_item(q, 1, i) for i in range(4 * q + 4)]
                if q == 3:
                    extra = [(14, pv_item(3, 1, 0, 0)),
                             (14, pv_item(3, 1, 0, 1)),
                             (15, pv_item(3, 1, 1, 0)),
                             (15, pv_item(3, 1, 1, 1)),
                             (15, tr_item(3, 1, 0))]
                    weave(mastersB, FB[q], extra)
                else:
                    weave(mastersB, FB[q])

            # tail: remaining PV(3,1), transposes, c(3,*) staggered
            pv_item(3, 1, 2, 0)[0]()
            pv_item(3, 1, 2, 1)[0]()
            tr_item(3, 1, 1)[0]()
            c_item(3, 0, 0)[0]()
            c_item(3, 0, 1)[0]()
            pv_item(3, 1, 3, 0)[0]()
            pv_item(3, 1, 3, 1)[0]()
            tr_item(3, 1, 2)[0]()
            c_item(3, 1, 0)[0]()
            c_item(3, 1, 1)[0]()
            tr_item(3, 1, 3)[0]()
            c_item(3, 2, 0)[0]()
            c_item(3, 2, 1)[0]()
            c_item(3, 3, 0)[0]()
            c_item(3, 3, 1)[0]()

    nc.compile()
    return nc


def _get_nc():
    if "nc" not in _CACHE:
        _CACHE["nc"] = _build()
    return _CACHE["nc"]


def _get_runner():
    """Build the jitted SPMD executor once (mirrors bass2jax.run_bass_via_pjrt
    but caches the jitted function so repeat calls skip retrace/recompile)."""
    if "runner" in _CACHE:
        return _CACHE["runner"]
    import jax
    import numpy as _np
    from jax.sharding import Mesh, PartitionSpec
    from jax.experimental.shard_map import shard_map
    import concourse.mybir as mybir
    from concourse import bass2jax

    nc = _get_nc()
    bass2jax.install_neuronx_cc_hook()

    partition_name = (nc.partition_id_tensor.name
                      if nc.partition_id_tensor else None)
    in_names, out_names, out_avals, zero_shapes = [], [], [], []
    for alloc in nc.m.functions[0].allocations:
        if not isinstance(alloc, mybir.MemoryLocationSet):
            continue
        name = alloc.memorylocations[0].name
        if alloc.kind == "ExternalInput":
            if name != partition_name:
                in_names.append(name)
        elif alloc.kind == "ExternalOutput":
            out_avals.append(jax.core.ShapedArray(
                tuple(alloc.tensor_shape), mybir.dt.np(alloc.dtype)))
            out_names.append(name)
            zero_shapes.append((tuple(alloc.tensor_shape),
                                mybir.dt.np(alloc.dtype)))
    n_params = len(in_names)
    n_outs = len(out_names)
    all_names = in_names + out_names
    if partition_name is not None:
        all_names = all_names + [partition_name]

    def _body(*args):
        operands = list(args)
        if partition_name is not None:
            operands.append(bass2jax.partition_id_tensor())
        outs = bass2jax._bass_exec_p.bind(
            *operands,
            out_avals=tuple(out_avals),
            in_names=tuple(all_names),
            out_names=tuple(out_names),
            lowering_input_output_aliases=(),
            sim_require_finite=True,
            sim_require_nnan=True,
            nc=nc,
        )
        return tuple(outs)

    devices = jax.devices()[:N_CORES]
    mesh = Mesh(_np.asarray(devices), ("core",))
    donate = tuple(range(n_params, n_params + n_outs))
    sharded = jax.jit(
        shard_map(_body, mesh=mesh,
                  in_specs=(PartitionSpec("core"),) * (n_params + n_outs),
                  out_specs=(PartitionSpec("core"),) * n_outs,
                  check_rep=False),
        donate_argnums=donate, keep_unused=True)

    def run(in_maps):
        concat_in = [
            _np.concatenate([_np.asarray(m[name]) for m in in_maps], axis=0)
            for name in in_names]
        concat_zeros = [
            _np.zeros((N_CORES * sh[0], *sh[1:]), dtype)
            for sh, dtype in zero_shapes]
        out_arrs = sharded(*concat_in, *concat_zeros)
        return [
            {name: _np.asarray(out_arrs[i]).reshape(
                N_CORES, *zero_shapes[i][0])[c]
             for i, name in enumerate(out_names)}
            for c in range(N_CORES)]

    _CACHE["runner"] = run
    return run


def _fp8():
    return (ml_dtypes.float8_e4m3fn if hasattr(ml_dtypes, 'float8_e4m3fn')
            else ml_dtypes.float8_e4m3)


def kernel(x, mask, W_qkv, b_qkv, W_out, b_out):
    bf = ml_dtypes.bfloat16
    f8 = _fp8()
    x = np.asarray(x, dtype=np.float32)
    W_qkv = np.asarray(W_qkv, dtype=np.float32)
    b_qkv = np.asarray(b_qkv, dtype=np.float32)
    W_out = np.asarray(W_out, dtype=np.float32)
    b_out = np.asarray(b_out, dtype=np.float32)
    # mask is the causal tril mask (per problem spec); causality is
    # implemented structurally on-device.

    run = _get_runner()

    def pack(wslice):
        # [C, DH] -> [P, KO, DH] with C = ko*P + p
        return np.ascontiguousarray(
            wslice.reshape(KO, P, DH).transpose(1, 0, 2))

    def w_hilo(wslice):
        # hi/lo fp8 split: 32W ~= Wh + (Wl term via xh) with W1 for xl term
        w32 = wslice * WSCALE
        wh = w32.astype(f8)
        wl = (w32 - wh.astype(np.float32)).astype(f8)
        w1 = wslice.astype(f8)
        return (pack(wh.astype(np.float32)).astype(f8),
                pack(w1.astype(np.float32)).astype(f8),
                pack(wl.astype(np.float32)).astype(f8))

    def pack_b(bslice):
        # [DH] -> [P, 2] with idx = s*P + p, x32
        return np.ascontiguousarray(
            (bslice * WSCALE).reshape(2, P).T).astype(np.float32)

    tri = np.triu(np.ones((P, P), dtype=np.float32))
    tri2 = np.ascontiguousarray(
        np.broadcast_to(tri[:, None, :], (P, 2, P))).astype(bf)
    ident = np.eye(P, dtype=np.float32).astype(bf)

    xhs, xls = [], []
    for b in range(B):
        xt = np.ascontiguousarray(x[b].T)
        xh = xt.astype(f8)
        xl = ((xt - xh.astype(np.float32)) * WSCALE).astype(f8)
        xhs.append(xh)
        xls.append(xl)

    in_maps = []
    for core in range(N_CORES):
        b, g = divmod(core, GROUPS)
        cs = slice(g * DH, (g + 1) * DH)
        wq3 = w_hilo(W_qkv[:, cs])
        wk3 = w_hilo(W_qkv[:, C:][:, cs])
        wv3 = w_hilo(W_qkv[:, 2 * C:][:, cs])
        in_maps.append({
            "xh": xhs[b], "xl": xls[b],
            "wqh": wq3[0], "wq1": wq3[1], "wql": wq3[2],
            "wkh": wk3[0], "wk1": wk3[1], "wkl": wk3[2],
            "wvh": wv3[0], "wv1": wv3[1], "wvl": wv3[2],
            "wo": np.ascontiguousarray(
                W_out[cs, :].reshape(2, P, C).transpose(1, 0, 2)).astype(bf),
            "bq": pack_b(b_qkv[cs]),
            "bk": pack_b(b_qkv[C:][cs]),
            "bv": np.ascontiguousarray(
                (b_qkv[2 * C:][cs] * WSCALE)[None, :]).astype(np.float32),
            "tri": tri2,
            "ident": ident,
        })

    results = run(in_maps)

    out = np.zeros((B, T, C), dtype=np.float32)
    for core in range(N_CORES):
        b = core // GROUPS
        out[b] += results[core]["out"].astype(np.float32)
    out += b_out[None, None, :]
    return out


# revision 39
# speedup vs baseline: 1.1273x; 1.0006x over previous
"""Trainium2 Bass kernel for nn_CausalSelfAttention (B=2, T=2048, C=1024, 16 heads).

Sharding: 8 cores = 2 batches x 4 head-groups (4 heads each).

v3 design (cost-model driven):
  - QKV projections run as fp8(e4m3) DoubleRow matmuls with hi/lo error
    compensation: 32*x@W = xh@fp8(32W) + fp8(32(x-xh))@fp8(W)
    + xh@fp8(32W - fp8(32W)), all three terms accumulated in one PSUM
    group. DR charges 0.5 cycles/output-col and packs 2 contraction
    chunks per instruction, so this costs 0.75x of bf16 at ~0.3% error
    (plain fp8 would be 0.25x cost but ~5% error -- softmax averaging
    shrinks signal and noise equally, so that error hits the output
    full-strength and blows the 2e-2 gate).
  - S = Q^T K, P (exp output), P@V, and the out-projection stay bf16.
  - exp runs on ACT (~73us busy); PE (~90us busy) is the bottleneck, the
    emission weaves S tiles and filler work by a cost ledger.
  - diagonal causal masking: post-exp multiply by triu-ones on the
    (otherwise idle) gpsimd/Pool engine.
  - O is normalized in [q,d] layout then transposed via PE (identity
    matmul) + DVE copy -- no DMA XBAR on the critical path.
  - out-projection in bf16, partial [T,C] per core; host sums the 4
    tensor-parallel partials per batch and adds b_out.
"""
import sys

if '/opt/trn_rl_repo' not in sys.path:
    sys.path.insert(0, '/opt/trn_rl_repo')

import numpy as np
import ml_dtypes

B, T, C = 2, 2048, 1024
N_HEAD = 16
D = 64
P = 128
N_CORES = 8
GROUPS = N_CORES // B            # 4 tensor-parallel groups per batch
HPC = N_HEAD // GROUPS           # 4 heads per core
DH = HPC * D                     # 256 head dims per core
KO = C // P                      # 8 contraction subtiles for projections
NQB = T // 512                   # 4 q blocks of 512
WSCALE = 32.0                    # fp8 range scaling of W_qkv
SCALE = 1.0 / (np.sqrt(D) * WSCALE * WSCALE)   # exp scale

_CACHE = {}

# weave pacing costs (ns)
PE_CYC = 0.4167


def _build():
    import concourse.mybir as mybir
    import concourse.tile as tile
    from concourse import bacc

    f32 = mybir.dt.float32
    bf16 = mybir.dt.bfloat16
    f16 = mybir.dt.float16
    fp8 = mybir.dt.float8e4
    DR = mybir.MatmulPerfMode.DoubleRow
    EXP = mybir.ActivationFunctionType.Exp
    MUL = mybir.AluOpType.mult
    ADD = mybir.AluOpType.add

    nc = bacc.Bacc("TRN2", target_bir_lowering=False, debug=False,
                   num_devices=N_CORES)

    xh_d = nc.dram_tensor("xh", [C, T], fp8, kind="ExternalInput")
    xl_d = nc.dram_tensor("xl", [C, T], fp8, kind="ExternalInput")
    w_ds = {}
    for nm in ("wqh", "wq1", "wql", "wkh", "wk1", "wkl",
               "wvh", "wv1", "wvl"):
        w_ds[nm] = nc.dram_tensor(nm, [P, KO, DH], fp8, kind="ExternalInput")
    wo_d = nc.dram_tensor("wo", [P, 2, C], bf16, kind="ExternalInput")
    bq_d = nc.dram_tensor("bq", [P, 2], f32, kind="ExternalInput")
    bk_d = nc.dram_tensor("bk", [P, 2], f32, kind="ExternalInput")
    bv_d = nc.dram_tensor("bv", [1, DH], f32, kind="ExternalInput")
    tri_d = nc.dram_tensor("tri", [P, 2, P], bf16, kind="ExternalInput")
    id_d = nc.dram_tensor("ident", [P, P], bf16, kind="ExternalInput")
    out_d = nc.dram_tensor("out", [T, C], f16, kind="ExternalOutput")

    pls = [slice(0, D), slice(D, 2 * D)]

    with tile.TileContext(nc) as tc:
        with (
            tc.tile_pool(name="pp", bufs=1) as pp,
            tc.tile_pool(name="wS", bufs=4) as wS,
            tc.tile_pool(name="wO", bufs=4) as wO,
            tc.tile_pool(name="psS", bufs=2, space="PSUM") as psS,
            tc.tile_pool(name="psU", bufs=4, space="PSUM") as psU,
        ):
            xhs_t = [pp.tile([P, KO, 512], fp8, tag=f"xh{q}", name=f"xh{q}")
                     for q in range(4)]
            xls_t = [pp.tile([P, KO, 512], fp8, tag=f"xl{q}", name=f"xl{q}")
                     for q in range(4)]

            def x_mv(xt_, q, t):
                # moving operand [p, 2(ko-pair), 512]
                return xt_[q][:, 2 * t:2 * t + 2, :]

            def x_st(xt_, q, t, it_):
                # stationary operand [p, 2(ko-pair), 128]
                return xt_[q][:, 2 * t:2 * t + 2, it_ * P:(it_ + 1) * P]
            wt = {nm: pp.tile([P, KO, DH], fp8, tag=nm, name=nm)
                  for nm in ("wqh", "wq1", "wql", "wkh", "wk1", "wkl",
                             "wvh", "wv1", "wvl")}
            wos = pp.tile([P, 2, C], bf16, tag="wos")
            qts = [[pp.tile([P, 512], bf16, tag=f"qt{s}_{q}", name=f"qt{s}_{q}")
                    for q in range(4)] for s in range(2)]
            kts = [[pp.tile([P, 512], bf16, tag=f"kt{s}_{q}", name=f"kt{s}_{q}")
                    for q in range(4)] for s in range(2)]
            # vo: [k-part, kt-slot, head(2hs+u), d + ones-col]
            vo = pp.tile([P, 16, HPC, D + 1], bf16, tag="vo")
            # pt: per hs P^T tiles [k-part, kt-slot, u, q-col of current block]
            pts = [pp.tile([P, 16, 2, 512], bf16, tag=f"pt{hs}", name=f"pt{hs}")
                   for hs in range(2)]
            ots = [[pp.tile([P, 512], bf16, tag=f"ot{j}_{hs}",
                            name=f"ot{j}_{hs}") for hs in range(2)]
                   for j in range(NQB)]
            bqs = pp.tile([P, 2], f32, tag="bqs")
            bks = pp.tile([P, 2], f32, tag="bks")
            bvrow = pp.tile([1, DH], f32, tag="bvrow")
            bvb = pp.tile([P, DH], f32, tag="bvb")
            trib = pp.tile([P, 2, P], bf16, tag="trib")
            ident = pp.tile([P, P], bf16, tag="ident")
            scr = pp.tile([1, 1], f32, tag="scr")

            # trigger the exp ACT-table load immediately (scratch memset)
            nc.vector.memset(scr[:], 0.0)
            nc.scalar.activation(scr[0:1, 0:1], scr[0:1, 0:1], EXP)
            # ones-column (=WSCALE) of V for softmax denominators
            nc.vector.memset(vo[:, :, :, D:D + 1], WSCALE)

            # ---- loads (sync=SP queue, scalar=ACT queue, gpsimd=SWDGE) ----
            xh_r = xh_d.rearrange("(ko p) t -> p ko t", p=P)
            xl_r = xl_d.rearrange("(ko p) t -> p ko t", p=P)
            nc.sync.dma_start(wt["wqh"][:], w_ds["wqh"][:])
            nc.scalar.dma_start(xhs_t[0][:], xh_r[:, :, 0:512])
            nc.sync.dma_start(wt["wq1"][:], w_ds["wq1"][:])
            nc.scalar.dma_start(xls_t[0][:], xl_r[:, :, 0:512])
            nc.sync.dma_start(wt["wql"][:], w_ds["wql"][:])
            nc.sync.dma_start(bqs[:], bq_d[:])
            for nm in ("wkh", "wk1", "wkl"):
                nc.gpsimd.dma_start(wt[nm][:], w_ds[nm][:])
            nc.sync.dma_start(bks[:], bk_d[:])
            for nm in ("wvh", "wv1", "wvl"):
                nc.sync.dma_start(wt[nm][:], w_ds[nm][:])
            nc.sync.dma_start(bvrow[:], bv_d[:])
            nc.sync.dma_start(trib[:], tri_d[:])
            nc.sync.dma_start(ident[:], id_d[:])
            for q in range(1, 4):
                nc.scalar.dma_start(xhs_t[q][:],
                                    xh_r[:, :, q * 512:(q + 1) * 512])
                nc.scalar.dma_start(xls_t[q][:],
                                    xl_r[:, :, q * 512:(q + 1) * 512])
            nc.sync.dma_start(wos[:], wo_d[:])

            nc.gpsimd.partition_broadcast(bvb[:, :], bvrow[0:1, :])

            # ---------- item constructors (thunk, pe_ns, act_ns) ----------
            # hi/lo fp8 compensation: 32xW = xh@Wh + xl@W1 + xh@Wl, all
            # accumulated in one PSUM group (12 DR steps).
            def qk8_item(q, wpfx, dsts, bias, s_):
                terms = [(wt[wpfx + "h"], xhs_t), (wt[wpfx + "1"], xls_t),
                         (wt[wpfx + "l"], xhs_t)]

                def it():
                    pq = psU.tile([P, 512], f32, tag="ps1",
                                  name=f"pq{q}_{wpfx}_{s_}")
                    for ti, (w8, xsrc) in enumerate(terms):
                        for t in range(KO // 2):
                            nc.tensor.matmul(
                                pq[:],
                                w8[:, 2 * t:2 * t + 2, s_ * P:(s_ + 1) * P],
                                x_mv(xsrc, q, t),
                                start=(ti == 0 and t == 0),
                                stop=(ti == 2 and t == KO // 2 - 1),
                                perf_mode=DR)
                    nc.vector.tensor_scalar_add(
                        dsts[s_][q][:], pq[:], bias[:, s_:s_ + 1])
                return (it, 1290, 0)

            def qk8_items(q):
                # order: Q s0, K s0, Q s1, K s1 (heads-split 0 first so the
                # next phase's S(q,0) stream unblocks earliest)
                return [qk8_item(q, "wq", qts, bqs, 0),
                        qk8_item(q, "wk", kts, bks, 0),
                        qk8_item(q, "wq", qts, bqs, 1),
                        qk8_item(q, "wk", kts, bks, 1)]

            def v8_item(q, it_):
                terms = [(wt["wvh"], xhs_t), (wt["wv1"], xls_t),
                         (wt["wvl"], xhs_t)]

                def it():
                    pv = psU.tile([P, DH], f32, tag="ps1",
                                  name=f"pv{q}_{it_}")
                    for ti, (w8, xsrc) in enumerate(terms):
                        for t in range(KO // 2):
                            nc.tensor.matmul(
                                pv[:],
                                x_st(xsrc, q, t, it_),
                                w8[:, 2 * t:2 * t + 2, :],
                                start=(ti == 0 and t == 0),
                                stop=(ti == 2 and t == KO // 2 - 1),
                                perf_mode=DR)
                    nc.vector.tensor_tensor(
                        vo[:, 4 * q + it_, :, 0:D],
                        pv[:].rearrange("p (h d) -> p h d", h=HPC),
                        bvb.rearrange("p (h d) -> p h d", h=HPC),
                        ADD)
                return (it, 645, 0)

            def s_item(q, hs, i):
                off = max(0, P * i - 512 * q)
                diag = P * i >= 512 * q

                def it():
                    sp = psS.tile([P, 2, 512], f32, tag="sp",
                                  name=f"sp{q}_{hs}_{i}")
                    for u in range(2):
                        nc.tensor.matmul(
                            sp[:, u, off:512],
                            kts[hs][i // 4][pls[u],
                                            (i % 4) * P:(i % 4 + 1) * P],
                            qts[hs][q][pls[u], off:512],
                            start=True, stop=True)
                    nc.scalar.activation(pts[hs][:, i, :, off:512],
                                         sp[:, :, off:512],
                                         EXP, scale=float(SCALE))
                    if diag:
                        nc.gpsimd.tensor_tensor(
                            pts[hs][:, i, :, off:off + P],
                            pts[hs][:, i, :, off:off + P],
                            trib[:], MUL)
                w = 512 - off
                return (it, 2 * w * PE_CYC + 20, 2 * w * 0.833 + 190)

            def pv_item(q, hs, qc, u):
                qt = 4 * q + qc

                def it():
                    po = psU.tile([P, 512], f32, tag="ps1",
                                  name=f"po{q}_{hs}_{qc}_{u}")
                    for kt in range(qt + 1):
                        nc.tensor.matmul(
                            po[:, 0:D + 1],
                            pts[hs][:, kt, u, qc * P:(qc + 1) * P],
                            vo[:, kt, 2 * hs + u, :],
                            start=(kt == 0), stop=(kt == qt))
                    rd = wS.tile([P, 1], f32, tag="rd",
                                 name=f"rd{q}_{hs}_{qc}_{u}")
                    if u == 0:
                        osbs[(q, hs, qc)] = wS.tile(
                            [P, 2, D], bf16, tag="osb",
                            name=f"osb{q}_{hs}_{qc}")
                    osb = osbs[(q, hs, qc)]
                    nc.vector.reciprocal_approx_fast(rd[:], po[:, D:D + 1])
                    nc.vector.tensor_scalar_mul(
                        osb[:, u, :], po[:, 0:D], rd[:])
                return (it, (qt + 1) * 27 + 30, 0)

            osbs = {}

            def tr_item(q, hs, qc):
                def it():
                    ptr = psU.tile([P, P], bf16, tag="ps1",
                                   name=f"ptr{q}_{hs}_{qc}")
                    nc.tensor.transpose(
                        ptr[:], osbs[(q, hs, qc)].rearrange("p u d -> p (u d)"),
                        ident[:])
                    nc.vector.tensor_copy(
                        ots[q][hs][:, qc * P:(qc + 1) * P], ptr[:])
                return (it, 75, 0)

            def c_item(jj, mo, n):
                def it():
                    pc = psU.tile([P, 512], f32, tag="ps1",
                                  name=f"pc{jj}_{mo}_{n}")
                    for s in range(2):
                        nc.tensor.matmul(
                            pc[:],
                            ots[jj][s][:, mo * P:(mo + 1) * P],
                            wos[:, s, n * 512:(n + 1) * 512],
                            start=(s == 0), stop=(s == 1))
                    if n == 0:
                        obs[(jj, mo)] = wO.tile([P, C], f16, tag="ob",
                                                name=f"ob{jj}_{mo}")
                    ob = obs[(jj, mo)]
                    nc.vector.tensor_copy(
                        ob[:, n * 512:(n + 1) * 512], pc[:])
                    m = 4 * jj + mo
                    nc.sync.dma_start(
                        out_d[m * P:(m + 1) * P, n * 512:(n + 1) * 512],
                        ob[:, n * 512:(n + 1) * 512])
                return (it, 430, 0)

            obs = {}

            # ---------- schedule ----------
            # master stream: all S tiles in (q, hs, i) order; ACT must never
            # starve, PE (the bottleneck) must never park behind a stalled
            # S matmul. Fillers are paced by a global PE-vs-ACT cost ledger;
            # per-phase filler assignment matches each segment's PE deficit
            # (~611ns per S tile).
            def weave(masters, fillers, extra=()):
                act_t, pe_t, fi = 0.0, 0.0, 0
                for k, (it, pe, act) in enumerate(masters):
                    it()
                    act_t += act
                    pe_t += pe
                    while fi < len(fillers) and pe_t + 400 < act_t:
                        f, fpe, _ = fillers[fi]
                        f()
                        pe_t += fpe
                        fi += 1
                    for pos, item in extra:
                        if pos == k:
                            item[0]()
                            pe_t += item[1]
                while fi < len(fillers):
                    f, fpe, _ = fillers[fi]
                    f()
                    fi += 1

            def pv_pair(q, hs, qc):
                return [pv_item(q, hs, qc, 0), pv_item(q, hs, qc, 1)]

            # prologue: q=0 projections at term granularity so PE starts as
            # soon as the first weight chunk lands
            def qk_term(q, wpfx, s_, ti, dsts=None, bias=None):
                nms = (wpfx + "h", wpfx + "1", wpfx + "l")
                xsrcs = (xhs_t, xls_t, xhs_t)
                key = ("pro", q, wpfx, s_)

                def it():
                    if ti == 0:
                        pro_ps[key] = psU.tile([P, 512], f32, tag="ps1",
                                               name=f"pq{q}_{wpfx}_{s_}")
                    pq = pro_ps[key]
                    w8 = wt[nms[ti]]
                    for t in range(KO // 2):
                        nc.tensor.matmul(
                            pq[:],
                            w8[:, 2 * t:2 * t + 2, s_ * P:(s_ + 1) * P],
                            x_mv(xsrcs[ti], q, t),
                            start=(ti == 0 and t == 0),
                            stop=(ti == 2 and t == KO // 2 - 1),
                            perf_mode=DR)
                    if ti == 2:
                        nc.vector.tensor_scalar_add(
                            dsts[s_][q][:], pq[:], bias[:, s_:s_ + 1])
                return (it, 430, 0)

            pro_ps = {}
            for ti in range(3):
                qk_term(0, "wq", 0, ti, qts, bqs)[0]()
                qk_term(0, "wk", 0, ti, kts, bks)[0]()

            masters0 = [s_item(0, 0, i) for i in range(4)]
            fillers0 = [qk8_item(0, "wq", qts, bqs, 1),
                        qk8_item(0, "wk", kts, bks, 1)]
            weave(masters0, fillers0)

            # per-phase filler assignment (see header comment)
            FA = {1: [], 2: [], 3: []}
            FB = {0: [], 1: [], 2: [], 3: []}
            # phase 0 B: v8(0), PV(0,0), tr, A(1), v8(1)
            FB[0] += [v8_item(0, 0), v8_item(0, 1)]
            for qc in range(4):
                FB[0] += pv_pair(0, 0, qc)
                if qc == 0:
                    FB[0] += [v8_item(0, 2), v8_item(0, 3)]
                if qc >= 1:
                    FB[0].append(tr_item(0, 0, qc - 1))
            FB[0].append(tr_item(0, 0, 3))
            FB[0] += qk8_items(1)
            FB[0] += [v8_item(1, k) for k in range(4)]
            # phase 1 A: PV(0,1), tr, A(2) s0-half, v8(2)
            for qc in range(4):
                FA[1] += pv_pair(0, 1, qc)
                if qc >= 1:
                    FA[1].append(tr_item(0, 1, qc - 1))
            FA[1].append(tr_item(0, 1, 3))
            FA[1] += qk8_items(2)[0:2]
            FA[1] += [v8_item(2, k) for k in range(4)]
            # phase 1 B: PV(1,0), tr, A(2) s1-half
            for qc in range(4):
                FB[1] += pv_pair(1, 0, qc)
                if qc >= 1:
                    FB[1].append(tr_item(1, 0, qc - 1))
            FB[1].append(tr_item(1, 0, 3))
            FB[1] += qk8_items(2)[2:4]
            # phase 2 A: PV(1,1), tr, A(3) s0-half, v8(3)
            for qc in range(4):
                FA[2] += pv_pair(1, 1, qc)
                if qc >= 1:
                    FA[2].append(tr_item(1, 1, qc - 1))
            FA[2].append(tr_item(1, 1, 3))
            FA[2] += qk8_items(3)[0:2]
            FA[2] += [v8_item(3, k) for k in range(4)]
            # phase 2 B: PV(2,0), tr, A(3) s1-half
            for qc in range(4):
                FB[2] += pv_pair(2, 0, qc)
                if qc >= 1:
                    FB[2].append(tr_item(2, 0, qc - 1))
            FB[2].append(tr_item(2, 0, 3))
            FB[2] += qk8_items(3)[2:4]
            # phase 3 A: PV(2,1), tr, c(1) all, c(2) all
            for qc in range(4):
                FA[3] += pv_pair(2, 1, qc)
                if qc >= 1:
                    FA[3].append(tr_item(2, 1, qc - 1))
            FA[3].append(tr_item(2, 1, 3))
            for mo in range(4):
                FA[3] += [c_item(1, mo, 0), c_item(1, mo, 1)]
            for mo in range(4):
                FA[3] += [c_item(2, mo, 0), c_item(2, mo, 1)]
            # phase 3 B: PV(3,0), tr, c(0) all; PV(3,1,0..2) placed late
            # via `extra` (they need the last exps of this segment)
            for qc in range(4):
                FB[3] += pv_pair(3, 0, qc)
                if qc >= 1:
                    FB[3].append(tr_item(3, 0, qc - 1))
            FB[3].append(tr_item(3, 0, 3))
            for mo in range(4):
                FB[3] += [c_item(0, mo, 0), c_item(0, mo, 1)]

            for q in range(4):
                if q > 0:
                    mastersA = [s_item(q, 0, i) for i in range(4 * q + 4)]
                    weave(mastersA, FA[q])
                mastersB = [s_item(q, 1, i) for i in range(4 * q + 4)]
                if q == 3:
                    extra = [(14, pv_item(3, 1, 0, 0)),
                             (14, pv_item(3, 1, 0, 1)),
                             (15, pv_item(3, 1, 1, 0)),
                             (15, pv_item(3, 1, 1, 1)),
                             (15, tr_item(3, 1, 0))]
                    weave(mastersB, FB[q], extra)
                else:
                    weave(mastersB, FB[q])

            # tail: remaining PV(3,1), transposes, c(3,*) staggered
            pv_item(3, 1, 2, 0)[0]()
            pv_item(3, 1, 2, 1)[0]()
            tr_item(3, 1, 1)[0]()
            c_item(3, 0, 0)[0]()
            c_item(3, 0, 1)[0]()
            pv_item(3, 1, 3, 0)[0]()
            pv_item(3, 1, 3, 1)[0]()
            tr_item(3, 1, 2)[0]()
            c_item(3, 1, 0)[0]()
            c_item(3, 1, 1)[0]()
            tr_item(3, 1, 3)[0]()
            c_item(3, 2, 0)[0]()
            c_item(3, 2, 1)[0]()
            c_item(3, 3, 0)[0]()
            c_item(3, 3, 1)[0]()

    nc.compile()
    return nc


def _get_nc():
    if "nc" not in _CACHE:
        _CACHE["nc"] = _build()
    return _CACHE["nc"]


def _get_runner():
    """Build the jitted SPMD executor once (mirrors bass2jax.run_bass_via_pjrt
    but caches the jitted function so repeat calls skip retrace/recompile)."""
    if "runner" in _CACHE:
        return _CACHE["runner"]
    import jax
    import numpy as _np
    from jax.sharding import Mesh, PartitionSpec
    from jax.experimental.shard_map import shard_map
    import concourse.mybir as mybir
    from concourse import bass2jax

    nc = _get_nc()
    bass2jax.install_neuronx_cc_hook()

    partition_name = (nc.partition_id_tensor.name
                      if nc.partition_id_tensor else None)
    in_names, out_names, out_avals, zero_shapes = [], [], [], []
    for alloc in nc.m.functions[0].allocations:
        if not isinstance(alloc, mybir.MemoryLocationSet):
            continue
        name = alloc.memorylocations[0].name
        if alloc.kind == "ExternalInput":
            if name != partition_name:
                in_names.append(name)
        elif alloc.kind == "ExternalOutput":
            out_avals.append(jax.core.ShapedArray(
                tuple(alloc.tensor_shape), mybir.dt.np(alloc.dtype)))
            out_names.append(name)
            zero_shapes.append((tuple(alloc.tensor_shape),
                                mybir.dt.np(alloc.dtype)))
    n_params = len(in_names)
    n_outs = len(out_names)
    all_names = in_names + out_names
    if partition_name is not None:
        all_names = all_names + [partition_name]

    def _body(*args):
        operands = list(args)
        if partition_name is not None:
            operands.append(bass2jax.partition_id_tensor())
        outs = bass2jax._bass_exec_p.bind(
            *operands,
            out_avals=tuple(out_avals),
            in_names=tuple(all_names),
            out_names=tuple(out_names),
            lowering_input_output_aliases=(),
            sim_require_finite=True,
            sim_require_nnan=True,
            nc=nc,
        )
        return tuple(outs)

    devices = jax.devices()[:N_CORES]
    mesh = Mesh(_np.asarray(devices), ("core",))
    donate = tuple(range(n_params, n_params + n_outs))
    sharded = jax.jit(
        shard_map(_body, mesh=mesh,
                  in_specs=(PartitionSpec("core"),) * (n_params + n_outs),
                  out_specs=(PartitionSpec("core"),) * n_outs,
                  check_rep=False),
        donate_argnums=donate, keep_unused=True)

    def run(in_maps):
        concat_in = [
            _np.concatenate([_np.asarray(m[name]) for m in in_maps], axis=0)
            for name in in_names]
        concat_zeros = [
            _np.zeros((N_CORES * sh[0], *sh[1:]), dtype)
            for sh, dtype in zero_shapes]
        out_arrs = sharded(*concat_in, *concat_zeros)
        return [
            {name: _np.asarray(out_arrs[i]).reshape(
                N_CORES, *zero_shapes[i][0])[c]
             for i, name in enumerate(out_names)}
            for c in range(N_CORES)]

    _CACHE["runner"] = run
    return run


def _fp8():
    return (ml_dtypes.float8_e4m3fn if hasattr(ml_dtypes, 'float8_e4m3fn')
            else ml_dtypes.float8_e4m3)


def kernel(x, mask, W_qkv, b_qkv, W_out, b_out):
    bf = ml_dtypes.bfloat16
    f8 = _fp8()
    x = np.asarray(x, dtype=np.float32)
    W_qkv = np.asarray(W_qkv, dtype=np.float32)
    b_qkv = np.asarray(b_qkv, dtype=np.float32)
    W_out = np.asarray(W_out, dtype=np.float32)
    b_out = np.asarray(b_out, dtype=np.float32)
    # mask is the causal tril mask (per problem spec); causality is
    # implemented structurally on-device.

    run = _get_runner()

    def pack(wslice):
        # [C, DH] -> [P, KO, DH] with C = ko*P + p
        return np.ascontiguousarray(
            wslice.reshape(KO, P, DH).transpose(1, 0, 2))

    def w_hilo(wslice):
        # hi/lo fp8 split: 32W ~= Wh + (Wl term via xh) with W1 for xl term
        w32 = wslice * WSCALE
        wh = w32.astype(f8)
        wl = (w32 - wh.astype(np.float32)).astype(f8)
        w1 = wslice.astype(f8)
        return (pack(wh.astype(np.float32)).astype(f8),
                pack(w1.astype(np.float32)).astype(f8),
                pack(wl.astype(np.float32)).astype(f8))

    def pack_b(bslice):
        # [DH] -> [P, 2] with idx = s*P + p, x32
        return np.ascontiguousarray(
            (bslice * WSCALE).reshape(2, P).T).astype(np.float32)

    tri = np.triu(np.ones((P, P), dtype=np.float32))
    tri2 = np.ascontiguousarray(
        np.broadcast_to(tri[:, None, :], (P, 2, P))).astype(bf)
    ident = np.eye(P, dtype=np.float32).astype(bf)

    xhs, xls = [], []
    for b in range(B):
        xt = np.ascontiguousarray(x[b].T)  # [C, T]
        xh = xt.astype(f8)
        xl = ((xt - xh.astype(np.float32)) * WSCALE).astype(f8)
        xhs.append(xh)
        xls.append(xl)

    in_maps = []
    for core in range(N_CORES):
        b, g = divmod(core, GROUPS)
        cs = slice(g * DH, (g + 1) * DH)
        wq3 = w_hilo(W_qkv[:, cs])
        wk3 = w_hilo(W_qkv[:, C:][:, cs])
        wv3 = w_hilo(W_qkv[:, 2 * C:][:, cs])
        in_maps.append({
            "xh": xhs[b], "xl": xls[b],
            "wqh": wq3[0], "wq1": wq3[1], "wql": wq3[2],
            "wkh": wk3[0], "wk1": wk3[1], "wkl": wk3[2],
            "wvh": wv3[0], "wv1": wv3[1], "wvl": wv3[2],
            "wo": np.ascontiguousarray(
                W_out[cs, :].reshape(2, P, C).transpose(1, 0, 2)).astype(bf),
            "bq": pack_b(b_qkv[cs]),
            "bk": pack_b(b_qkv[C:][cs]),
            "bv": np.ascontiguousarray(
                (b_qkv[2 * C:][cs] * WSCALE)[None, :]).astype(np.float32),
            "tri": tri2,
            "ident": ident,
        })

    results = run(in_maps)

    out = np.zeros((B, T, C), dtype=np.float32)
    for core in range(N_CORES):
        b = core // GROUPS
        out[b] += results[core]["out"].astype(np.float32)
    out += b_out[None, None, :]
    return out
